# revision 29
# baseline (speedup 1.0000x reference)
"""GQA attention prefill (B=2, S=1024, D=4096, H=32, KVH=8, HD=128) on 8 TRN2
NeuronCores.

Sharding: tensor-parallel over heads. Core c owns KV head c and query heads
4c..4c+3 (GQA groups align with cores), i.e. column-shards of wq/wk/wv and the
matching row-shard of wo. Each core computes its partial `attn_c @ wo_c`
([B*S, D]); the host sums the 8 partials.

Device inputs are host-pretiled into exact SBUF layouts so every DMA reads
fully contiguous per-partition lines (see _tile_* helpers):
  xh   [128, B*nkb, dc, 128]  bf16  activation half-chunks (contraction on
                                    partitions, 128 tokens per chunk)
  wq   [128, NH, dc, HD]  bf16  rope-permuted (even dims then odd, per head)
  wk   [128, dc, HD]      bf16  rope-permuted
  wv   [128, dc, HD]      bf16
  wo   [128, D/512, NH, 512] bf16
  sw   [128, 128]         bf16  identity (PE-transpose operand for V)
  c2   [128, S]   f32   rope cos table, duplicated across the two 64-row halves
  s2   [128, S]   f32   rope sin table, [-sin; +sin]
  mt   [128,128]  f32   causal diagonal-block additive mask * sqrt(HD)   (causal)
  mt   [S, S]     bf16  full transposed additive mask * sqrt(HD)         (general)

Q/K are produced transposed ([d, tok]) straight out of the projection matmul;
scores are computed transposed ([k, q]) so P^T feeds wide-N PV matmuls
(nested causal ranges) with no transposes anywhere. Softmax denominators:
DVE folds each PAIR of k-blocks of P^T into one tile (halving the columns
the slow [1,N]-shaped ones-matmuls stream through the PE), the per-pair
ones-matmuls accumulate the denominator row in PSUM two chunks late (so they
never head-of-line-block the PE on exp/fold latency), and a separate "den"
unit — placed so zipped filler matmuls run between it and the scores —
evicts the row, broadcasts it across partitions with a rank-1 PE matmul
(gpsimd's partition_broadcast pays a ~3us ucode-reload when alternating with
SWDGE work), and takes the DVE reciprocal. V is projected transposed and
PE-transposed back. Rope's even/odd pairing becomes a contiguous
partition-half swap by permuting the weight columns; the swap is two
SBUF->SBUF DMAs, and the swap/mul/add chain runs at 256-column grain over
per-128 PSUM evictions (interleaving accumulation groups of two column
regions in one PSUM tile diverges on hardware). Softmax skips the
max-subtraction (scores are O(10); exp accumulates in fp32).

Startup: the first matmul needs only half of wk + the first x half-chunk.
x and projection-weight DMAs ride the sync (HWDGE) ring in exact consumption
order (wk.0 | x0 | wk.1, wv | x1 | wq0 | x2 | wq1 | x3 | wq2, wq3 | x4..):
the Tile list scheduler orders ready DMAs by emission priority, so lazy loads
are emitted inside the first unit that needs them, and the big rope tables
are gated behind the first rope eviction (a 1-element WAW write) to keep
them out of the opening window. The first four half-chunks' units are
emitted in a hand-crafted order matched to DMA arrival. wo rides the scalar
ring, gated on the batch's last qT write so the scheduler cannot hoist the
(dependency-free) loads into the startup window. Output stores ride the two
HWDGE rings only.

Scheduling: all work is emitted as unit-closures and "zipped" — batch 1's
projections (at 256-token grain, so the zipped rope/eviction chains stay
coarse) interleave batch 0's attention, batch 0's output projection
interleaves batch 1's attention — so the in-order PE queue always holds
dense matmuls while exp/softmax chains drain on the other engines. PSUM
evictions are emitted one projection late to avoid head-of-line blocking.
"""

import math
from contextlib import ExitStack

import numpy as np
import ml_dtypes

import concourse.bass as bass
import concourse.bass_isa as bass_isa
import concourse.mybir as mybir
import concourse.tile as tile
from concourse import bacc
from concourse.bass_utils import run_bass_kernel_spmd

BF16 = mybir.dt.bfloat16
F32 = mybir.dt.float32
NPBF16 = ml_dtypes.bfloat16

B, S, D, H, KVH, HD = 2, 1024, 4096, 32, 8, 128
NCORES = 8
NH = H // NCORES          # 4 query heads per core
DC = D // 128             # 32 contraction chunks
SQ = 1.0 / math.sqrt(HD)


def _chunks(q0, qend, step=512):
    qs = q0
    while qs < qend:
        nq = min(step, qend - qs)
        yield qs, nq
        qs += nq


def build_program(causal, s=S, d=D):
    """Build the per-core SPMD program. s/d are overridable for sim tests."""
    dc = d // 128
    nkb = s // 128            # number of 128-wide key/query/token blocks per batch
    qcols = NH * HD

    # pT packing offsets: causal keeps only k-block ki's valid q range [128ki, s)
    if causal:
        q0s = [ki * 128 for ki in range(nkb)]
    else:
        q0s = [0] * nkb
    offs, acc = [], 0
    for ki in range(nkb):
        offs.append(acc)
        acc += s - q0s[ki]
    pt_len = acc

    nc = bacc.Bacc(
        "TRN2",
        target_bir_lowering=False,
        debug=False,
        enable_asserts=False,
        num_devices=1,
    )
    # all inputs are host-pretiled into the exact SBUF layouts, so every DMA
    # below reads fully contiguous per-partition lines
    xh = nc.dram_tensor("xh", [128, B * nkb, dc, 128], BF16, kind="ExternalInput").ap()
    wq = nc.dram_tensor("wq", [128, NH, dc, HD], BF16, kind="ExternalInput").ap()
    wk = nc.dram_tensor("wk", [128, dc, HD], BF16, kind="ExternalInput").ap()
    wv = nc.dram_tensor("wv", [128, dc, HD], BF16, kind="ExternalInput").ap()
    wo = nc.dram_tensor("wo", [128, d // 512, NH, 512], BF16, kind="ExternalInput").ap()
    sw = nc.dram_tensor("sw", [128, 128], BF16, kind="ExternalInput").ap()
    cs = nc.dram_tensor("cs", [128, 2 * s], BF16, kind="ExternalInput").ap()
    if causal:
        mt = nc.dram_tensor("mt", [128, 128], F32, kind="ExternalInput").ap()
    else:
        mt = nc.dram_tensor("mt", [s, s], BF16, kind="ExternalInput").ap()
    out = nc.dram_tensor("out", [B * s, d], BF16, kind="ExternalOutput").ap()

    with tile.TileContext(nc) as tc:
        with ExitStack() as ctx:
            const = ctx.enter_context(tc.tile_pool(name="const", bufs=1))
            xpool = ctx.enter_context(tc.tile_pool(name="xpool", bufs=3))
            wopool = ctx.enter_context(tc.tile_pool(name="wopool", bufs=2))
            qkv = ctx.enter_context(tc.tile_pool(name="qkv", bufs=2))
            ptp = ctx.enter_context(tc.tile_pool(name="ptp", bufs=2))
            rp = ctx.enter_context(tc.tile_pool(name="rp", bufs=2))
            small = ctx.enter_context(tc.tile_pool(name="small", bufs=2))
            tre = ctx.enter_context(tc.tile_pool(name="tre", bufs=2)) if causal else None
            oev = ctx.enter_context(tc.tile_pool(name="oev", bufs=2))
            # PSUM: psm (projections/oproj/V-transposes) and psA (attention
            # scores/PV/den-broadcast) are separate pools so a score matmul
            # never waits on a projection eviction queued behind exps on ACT
            # (and vice versa). 3 + 3 banks + 2 for the sums row = 8.
            psm = ctx.enter_context(tc.tile_pool(name="psm", bufs=3, space="PSUM"))
            psA = ctx.enter_context(tc.tile_pool(name="psA", bufs=3, space="PSUM"))
            psd = ctx.enter_context(tc.tile_pool(name="psd", bufs=1, space="PSUM"))

            # Startup streaming is split across BOTH HWDGE rings: the x
            # stream (+wv) rides sync, wk/wq ride scalar (idle at startup).
            # Per-queue throughput is ~550ns per per-partition LINE (16 DMA
            # engines): 2KB lines crawl at ~55GB/s while >=8KB lines stream
            # at full rate — so transfers are never split below 4KB lines,
            # and multi-chunk transfers are fused to widen the lines.
            # wk's first half is issued here; the second half is emitted
            # inside the first K unit.
            wk_sb = const.tile([128, dc, HD], BF16)
            nc.scalar.dma_start(wk_sb[:, 0 : dc // 2, :], wk[:, 0 : dc // 2, :])
            # small tables + the fused bf16 rope table ride the gpsimd queue
            # (separate SWDGE queue at ~75GB/s, out of the HWDGE rings' way)
            id_sb = const.tile([128, 128], BF16)
            nc.gpsimd.dma_start(id_sb[:], sw[:])
            if causal:
                mt_sb = const.tile([128, 128], F32)
                nc.gpsimd.dma_start(mt_sb[:], mt[:])
            else:
                mt_sb = const.tile([128, nkb, s], BF16)
                nc.gpsimd.dma_start(mt_sb[:], mt.rearrange("(kb p) q -> p kb q", p=128))
            # fused [cos | sin] table, bf16: half the bytes of the old f32
            # pair, one 4KB-line DMA, loaded up-front (first rope mul needs
            # it ~16us in; the old lazy f32 load arrived ~9us too late and
            # back-pressured the projection PSUM pool through the rope chain)
            cs_sb = const.tile([128, 2 * s], BF16)
            nc.gpsimd.dma_start(cs_sb[:], cs[:])
            # wv/wq are loaded lazily inside the first units that need them,
            # interleaved into their queues in exact consumption order.
            wv_sb = const.tile([128, dc, HD], BF16)
            wq_sb = const.tile([128, NH, dc, HD], BF16)
            loaded = set()
            ones_sb = const.tile([128, 1], BF16)
            nc.vector.memset(ones_sb[:], 1.0)
            onesr_sb = const.tile([1, 128], BF16)
            nc.vector.memset(onesr_sb[:], 1.0)

            def rope(pieces, tok0, w, out_slice):
                """pieces: per-128 psum tiles with raw projected Q/K columns
                (d-permuted). out = raw*c2 + halfswap(raw)*s2, written as bf16
                to out_slice. Evictions stay at 128-column PSUM grain; the
                swap / mul / add chain runs once per w columns."""
                raw = rp.tile([128, w], BF16, tag="raw", name=f"raw_{tok0}")
                for i, ps in enumerate(pieces):
                    nc.scalar.copy(raw[:, i * 128 : (i + 1) * 128], ps[:])
                swt = rp.tile([128, w], BF16, tag="swt", name=f"swt_{tok0}")
                nc.sync.dma_start(swt[0:64, :], raw[64:128, :])
                nc.sync.dma_start(swt[64:128, :], raw[0:64, :])
                t1 = rp.tile([128, w], F32, tag="t1", name=f"t1_{tok0}")
                nc.vector.tensor_mul(t1[:], swt[:], cs_sb[:, s + tok0 : s + tok0 + w])
                t2 = rp.tile([128, w], F32, tag="t2", name=f"t2_{tok0}")
                nc.vector.tensor_mul(t2[:], raw[:], cs_sb[:, tok0 : tok0 + w])
                nc.gpsimd.tensor_add(out_slice, t2[:], t1[:])

            def phase2(b, grain=1):
                """Stream x half-chunks, project Q/K/V for batch b. Returns
                the per-batch activation tiles. grain = number of 128-token
                halves per unit: batch 0 runs grain=1 for a fine-grained
                startup ramp; batch 1 runs grain=2 so the zipped filler units
                (and their rope/swap chains) stay coarse like the PE work."""
                qT_sb = qkv.tile([128, NH, s], BF16, tag="qT", name=f"qT_{b}")
                kT_sb = qkv.tile([128, s], BF16, tag="kT", name=f"kT_{b}")
                vT_sb = qkv.tile([128, s], BF16, tag="vT", name=f"vT_{b}")
                v_sb = qkv.tile([128, nkb, HD], BF16, tag="v", name=f"v_{b}")
                attnT_sb = qkv.tile([128, NH, s], BF16, tag="attnT", name=f"attnT_{b}")

                # evictions/rope are emitted one projection late, so each
                # engine's FIFO only sees work whose PSUM inputs are (nearly)
                # ready — avoids head-of-line blocking behind matmul chains.
                pending = []

                def flush(keep):
                    while len(pending) > keep:
                        kind, pieces, tok0_, w_ = pending.pop(0)
                        if kind == "k":
                            rope(pieces, tok0_, w_, kT_sb[:, tok0_ : tok0_ + w_])
                        elif kind.startswith("q"):
                            h = int(kind[1:])
                            rope(pieces, tok0_, w_, qT_sb[:, h, tok0_ : tok0_ + w_])
                        else:  # vt
                            for m2 in range(w_ // 128):
                                kb = tok0_ // 128 + m2
                                nc.vector.tensor_copy(
                                    vT_sb[:, kb * 128 : (kb + 1) * 128], pieces[m2][:]
                                )
                                vtp = psm.tile(
                                    [128, HD], BF16, tag="mm", name=f"vtp_{b}_{kb}"
                                )
                                nc.tensor.transpose(
                                    vtp[:], vT_sb[:, kb * 128 : (kb + 1) * 128], id_sb[:]
                                )
                                nc.scalar.copy(v_sb[:, kb, :], vtp[:])

                xcs = {}

                def load_x(js):
                    for j in js:
                        if j not in xcs:
                            if b == 0 and j <= 1 and nkb >= 2:
                                # x0+x1 fused into ONE 16KB-line transfer —
                                # the queue hits full HBM rate, vs 8KB lines
                                # at ~210GB/s
                                xc2 = xpool.tile(
                                    [128, 2, dc, 128], BF16, tag="xc2", bufs=1,
                                    name=f"xc2_{b}",
                                )
                                nc.sync.dma_start(xc2[:], xh[:, 0:2, :, :])
                                xcs[0] = xc2[:, 0]
                                xcs[1] = xc2[:, 1]
                                continue
                            xc = xpool.tile(
                                [128, dc, 128], BF16, tag="xc", name=f"xc_{b}_{j}"
                            )
                            nc.sync.dma_start(xc[:], xh[:, b * nkb + j, :, :])
                            xcs[j] = xc

                def proj(w_sb, js, nm):
                    # one PSUM tile + one clean start/stop accumulation chain
                    # per 128-token half (interleaving groups in a shared
                    # tile diverges on hardware)
                    pieces = []
                    for j in js:
                        ps = psm.tile([128, 128], F32, tag="mm", name=f"{nm}_{b}_{j}")
                        for c in range(dc):
                            nc.tensor.matmul(
                                ps[:],
                                w_sb[:, c, :],
                                xcs[j][:, c, :],
                                start=(c == 0),
                                stop=(c == dc - 1),
                            )
                        pieces.append(ps)
                    return pieces

                def mk_k(js):
                    def u():
                        load_x(js)
                        if b == 0 and "wk2" not in loaded:
                            loaded.add("wk2")
                            nc.scalar.dma_start(
                                wk_sb[:, dc // 2 :, :], wk[:, dc // 2 :, :]
                            )
                        pending.append(("k", proj(wk_sb, js, "kps"), js[0] * 128, 128 * len(js)))
                        flush(1)
                    return u

                def mk_vt(js):
                    def u():
                        # V projection, transposed like K (wide-N matmuls),
                        # then PE-transposed back to natural [tok, d] layout
                        if b == 0 and "wv" not in loaded:
                            loaded.add("wv")
                            nc.sync.dma_start(wv_sb[:], wv[:])
                        pending.append(("vt", proj(wv_sb, js, "vps"), js[0] * 128, 128 * len(js)))
                        flush(1)
                    return u

                def mk_q(js, h):
                    def u():
                        if b == 0 and f"wq{h}" not in loaded:
                            loaded.add(f"wq{h}")
                            nc.scalar.dma_start(wq_sb[:, h, :, :], wq[:, h, :, :])
                        pending.append((f"q{h}", proj(wq_sb[:, h], js, f"qps{h}"), js[0] * 128, 128 * len(js)))
                        flush(1)
                    return u

                mk = {"k": mk_k, "v": mk_vt}

                units = []
                if b == 0 and nkb >= 4:
                    # hand-crafted order for the first four half-chunks,
                    # matched to the dual-ring DMA arrival order:
                    #   sync:   [x0|x1] fused | wv | x2 | x3 ...
                    #   scalar: wk(halves) | wq0 | wq1 | wq2 | wq3 | wo..
                    #   gpsimd: id | mt | cs
                    crafted = [
                        (0, "k"), (1, "k"), (0, "q0"), (1, "q0"), (0, "v"),
                        (1, "v"), (0, "q1"), (1, "q1"), (2, "k"), (2, "v"),
                        (2, "q0"), (3, "k"), (0, "q2"), (1, "q2"), (3, "v"),
                        (2, "q1"), (3, "q0"), (0, "q3"), (1, "q3"), (2, "q2"),
                        (3, "q1"), (2, "q3"), (3, "q2"), (3, "q3"),
                    ]
                    for j, kind in crafted:
                        if kind.startswith("q"):
                            units.append(mk_q([j], int(kind[1:])))
                        else:
                            units.append(mk[kind]([j]))
                    first = 4
                else:
                    first = 0
                for j0 in range(first, nkb, grain):
                    js = list(range(j0, min(j0 + grain, nkb)))
                    units.append(mk_k(js))
                    units.append(mk_vt(js))
                    for h in range(NH):
                        units.append(mk_q(js, h))
                units.append(lambda: flush(0))
                T = dict(qT=qT_sb, kT=kT_sb, vT=vT_sb, v=v_sb, attnT=attnT_sb)
                return T, units

            def attn(b, T):
                """Attention units for batch b, emitted FINE-GRAINED: one
                unit per k-block pair of scores, one per PV q-chunk, one for
                the denominator tail. The zip weaves filler matmul units
                between every one of them, so the exp-gated stretches never
                head-of-line-block the in-order PE queue. PV of head h-1
                interleaves the scores of head h."""
                qT_sb, kT_sb, v_sb, attnT_sb = T["qT"], T["kT"], T["v"], T["attnT"]
                nsub = (nkb + 1) // 2          # scores sub-units per head
                ngroups = nsub if causal else nkb  # sums accumulation groups
                pv_chunks = list(_chunks(0, s))
                state = {}
                stage2 = {}

                def flush_sums(st, keep):
                    sums, psums = st["sums"], st["psums"]
                    while len(psums) > keep:
                        src, p_, a0, a1 = psums.pop(0)
                        nc.tensor.matmul(
                            sums[0:1, a0:a1],
                            ones_sb[:],
                            src(a0, a1),
                            start=(p_ == 0),
                            stop=(p_ == ngroups - 1),
                            skip_group_check=True,
                        )

                def scores_pair(h, p):
                    # scores + exp for k-blocks 2p, 2p+1. DVE folds the pair
                    # of k-blocks of pT into one tile (halving the columns
                    # the slow [1,N]-shaped denominator matmuls stream
                    # through the PE); the sums matmuls run two chunks late
                    # so they never wait on the exps or the fold.
                    if h not in state:
                        state[h] = dict(
                            pT=ptp.tile([128, pt_len], BF16, tag="pt", name=f"pt_{b}_{h}"),
                            sums=psd.tile([1, s], F32, tag="sums", bufs=1, name=f"sums_{b}_{h}"),
                            psums=[],
                        )
                    st = state[h]
                    pT = st["pT"]

                    def pslice(ki, qa, qb):
                        return pT[:, offs[ki] + qa - q0s[ki] : offs[ki] + qb - q0s[ki]]

                    for ki in (2 * p, 2 * p + 1):
                        if ki >= nkb:
                            continue
                        q0 = q0s[ki]
                        for qs_, nq in _chunks(q0, s):
                            sc = psA.tile([128, 512], F32, tag="sc", name=f"sc_{b}_{h}_{ki}_{qs_}")
                            nc.tensor.matmul(
                                sc[:, :nq],
                                kT_sb[:, ki * 128 : (ki + 1) * 128],
                                qT_sb[:, h, qs_ : qs_ + nq],
                                start=True,
                                stop=True,
                            )
                            if causal:
                                if qs_ == q0:  # diagonal block
                                    nc.vector.tensor_add(
                                        sc[:, 0:128], sc[:, 0:128], mt_sb[:]
                                    )
                            else:
                                nc.vector.tensor_add(
                                    sc[:, :nq], sc[:, :nq], mt_sb[:, ki, qs_ : qs_ + nq]
                                )
                            po = offs[ki] + qs_ - q0
                            nc.scalar.activation(
                                pT[:, po : po + nq],
                                sc[:, :nq],
                                mybir.ActivationFunctionType.Exp,
                                scale=SQ,
                            )
                            flush_sums(st, 2)
                        if not causal:
                            # general path: no pair-fold (SBUF for the full
                            # mask instead) — one ones-matmul per k-block
                            src = (lambda k: lambda qa, qb: pslice(k, qa, qb))(ki)
                            a0 = 0
                            while a0 < s:
                                a1 = min((a0 // 512 + 1) * 512, s)
                                st["psums"].append((src, ki, a0, a1))
                                a0 = a1
                            continue
                        if ki % 2 == 1 or ki == nkb - 1:
                            k0 = ki - 1 if ki % 2 == 1 else ki
                            qa0 = q0s[k0]
                            if ki % 2 == 1:
                                qb0 = q0s[ki]
                                s2 = tre.tile(
                                    [128, s - qa0], BF16, tag=f"s2_{p}",
                                    name=f"s2_{b}_{h}_{p}",
                                )
                                if qb0 > qa0:
                                    nc.vector.tensor_copy(
                                        s2[:, 0 : qb0 - qa0], pslice(k0, qa0, qb0)
                                    )
                                nc.vector.tensor_add(
                                    s2[:, qb0 - qa0 :],
                                    pslice(k0, qb0, s),
                                    pslice(ki, qb0, s),
                                )
                                src = (lambda t, o: lambda qa, qb: t[:, qa - o : qb - o])(s2, qa0)
                            else:  # odd tail block folds alone
                                src = (lambda k: lambda qa, qb: pslice(k, qa, qb))(ki)
                            # pieces split at 512-column boundaries so no
                            # matmul output crosses a PSUM bank
                            a0 = qa0
                            while a0 < s:
                                a1 = min((a0 // 512 + 1) * 512, s)
                                st["psums"].append((src, ki // 2, a0, a1))
                                a0 = a1
                    if p == nsub - 1:
                        flush_sums(st, 0)

                def attn_den(h):
                    # denominator tail: PSUM row eviction, rank-1 PE
                    # broadcast (gpsimd's partition_broadcast pays a ~3us
                    # ucode-reload when alternating with SWDGE work), then
                    # DVE reciprocal. A separate unit so zipped fillers sit
                    # between the scores and this chain.
                    st = state.pop(h)
                    pT, sums = st["pT"], st["sums"]
                    nhalf = (s + 511) // 512
                    width = s // nhalf
                    rbrs = []
                    for hs in range(nhalf):
                        rbr = small.tile([128, width], F32, tag="rbr", bufs=4 if causal else 2, name=f"rbr_{b}_{h}_{hs}")
                        ssb = small.tile([1, width], BF16, tag="ssb", bufs=4 if causal else 2, name=f"ssb_{b}_{h}_{hs}")
                        nc.scalar.copy(ssb[0:1, :], sums[0:1, hs * width : (hs + 1) * width])
                        rbp = psA.tile([128, 512], F32, tag="sc", name=f"rbp_{b}_{h}_{hs}")
                        nc.tensor.matmul(
                            rbp[:, :width], onesr_sb[:], ssb[0:1, :],
                            start=True, stop=True,
                        )
                        nc.vector.reciprocal_approx_fast(rbr[:], rbp[:, :width])
                        rbrs.append(rbr)
                    stage2[h] = (pT, rbrs, width)

                def attn_pv(h, ci):
                    # wide-N PV: per q-chunk, each k-block contributes one
                    # matmul over its (nested) valid q range, accumulating in
                    # PSUM — ki==0 always covers the whole chunk, so it opens
                    # the group for every column.
                    pT, rbrs, width = stage2[h]
                    qs0, w = pv_chunks[ci]
                    if ci == len(pv_chunks) - 1:
                        stage2.pop(h)
                    o_ps = psA.tile([128, 512], F32, tag="sc", name=f"ops_{b}_{h}_{ci}")
                    kis = [
                        k for k in range(nkb) if (not causal) or q0s[k] < qs0 + w
                    ]
                    for j, ki in enumerate(kis):
                        qlo = max(q0s[ki], qs0)
                        nc.tensor.matmul(
                            o_ps[:, qlo - qs0 : w],
                            v_sb[:, ki, :],
                            pT[:, offs[ki] + qlo - q0s[ki] : offs[ki] + qs0 + w - q0s[ki]],
                            start=(j == 0),
                            stop=(j == len(kis) - 1),
                            skip_group_check=True,
                        )
                    nc.vector.tensor_mul(
                        attnT_sb[:, h, qs0 : qs0 + w],
                        o_ps[:, :w],
                        rbrs[qs0 // width][:, qs0 % width : qs0 % width + w],
                    )

                units = []
                for h in range(NH):
                    for p in range(nsub):
                        units.append(lambda h=h, p=p: scores_pair(h, p))
                    units.append(lambda h=h: attn_den(h))
                    if h > 0:
                        for ci in range(len(pv_chunks)):
                            units.append(lambda h=h, ci=ci: attn_pv(h - 1, ci))
                for ci in range(len(pv_chunks)):
                    units.append(lambda ci=ci: attn_pv(NH - 1, ci))
                return units

            def oproj(b, T):
                """Output projection units (partial over this core's wo rows).
                wo loads ride the (idle) scalar queue so they never queue
                behind output-store DMAs. A one-element write gated on the
                batch's LAST qT rope pins each load's readiness past the
                startup DMA crunch — the Tile list scheduler would otherwise
                hoist these dependency-free loads to t=0 where they steal
                bandwidth from the opening wk/x transfers."""
                attnT_sb = T["attnT"]
                qT_sb = T["qT"]
                wo_nbs = {}

                def mk(nb, tp):
                    def u():
                        if tp == 0 and nb % 2 == 0:
                            # wo loads fused in PAIRS: 8KB lines stream at
                            # ~210GB/s vs 4KB lines at ~120 — halves the
                            # scalar-queue time and the ACT-engine issue count
                            wo2 = wopool.tile(
                                [128, 2, NH, 512], BF16, tag="wo", name=f"wo_{b}_{nb}"
                            )
                            nc.vector.tensor_copy(
                                wo2[0:1, 0, 0, 0:1], qT_sb[0:1, NH - 1, s - 1 : s]
                            )
                            nc.scalar.dma_start(wo2[:], wo[:, nb : nb + 2, :, :])
                            wo_nbs[nb] = wo2[:, 0]
                            wo_nbs[nb + 1] = wo2[:, 1]
                        ot = oev.tile([128, 2, 512], BF16, tag="ot", bufs=4 if causal else 2, name=f"ot_{b}_{nb}_{tp}")
                        for half in range(2):
                            tbk = tp * 2 + half
                            o2 = psm.tile([128, 512], F32, tag="mm", name=f"o2_{b}_{nb}_{tbk}")
                            for h in range(NH):
                                nc.tensor.matmul(
                                    o2[:],
                                    attnT_sb[:, h, tbk * 128 : (tbk + 1) * 128],
                                    wo_nbs[nb][:, h, :],
                                    start=(h == 0),
                                    stop=(h == NH - 1),
                                )
                            if half == 0:
                                nc.scalar.copy(ot[:, half, :], o2[:])
                            else:
                                nc.vector.tensor_copy(ot[:, half, :], o2[:])
                        # outs rotate over all three rings (gpsimd is idle by
                        # the oproj phases — its rope adds ended with the
                        # projections, and the softmax broadcast rides the PE
                        # — so SWDGE descgen there costs one ucode swap total)
                        ring = [nc.sync, nc.scalar, nc.gpsimd][(nb * 4 + tp) % 3]
                        ring.dma_start(
                            out[
                                b * s + tp * 256 : b * s + (tp + 1) * 256,
                                nb * 512 : (nb + 1) * 512,
                            ].rearrange("(rh p) n -> p rh n", p=128),
                            ot[:],
                        )
                    return u

                return [mk(nb, tp) for nb in range(d // 512) for tp in range(nkb // 2)]

            def zip_emit(primary, filler):
                """Emit primary units with filler units woven between them, so
                the in-order PE queue always has dense matmul work to run
                while the primary's cross-engine chains (exp/softmax) drain.
                Fractional weaving spreads the fillers over ALL primaries."""
                rate = len(filler) / max(1, len(primary))
                credit = 0.0
                fi = 0
                for i, u in enumerate(primary):
                    u()
                    credit += rate
                    while credit >= 1.0 and fi < len(filler):
                        filler[fi]()
                        fi += 1
                        credit -= 1.0
                # remaining fillers drain AFTER the last primary: draining
                # them first serializes their eviction/out-DMA chains in
                # front of the last primary's matmuls (measured ~6us PE gap
                # + a HAM re-throttle)
                while fi < len(filler):
                    filler[fi]()
                    fi += 1

            # batch 0 projections run alone; batch 0 attention is interleaved
            # with batch 1 projections; batch 1 attention with batch 0 output
            # projection; batch 1 output projection drains at the end.
            T0, p20 = phase2(0, grain=2)
            for u in p20:
                u()
            a0 = attn(0, T0)
            if B > 1:
                T1, p21 = phase2(1, grain=2)
                zip_emit(a0, p21)
                a1 = attn(1, T1)
                o0 = oproj(0, T0)
                zip_emit(a1, o0)
                for u in oproj(1, T1):
                    u()
            else:
                for u in a0:
                    u()
                for u in oproj(0, T0):
                    u()
    nc.compile()
    return nc


# ---------------------------------------------------------------------------
# host side
# ---------------------------------------------------------------------------

_PERM = np.concatenate([np.arange(0, HD, 2), np.arange(1, HD, 2)])
_CACHE = {}


def _tile_xh(x, s=S, d=D):
    """[B, s, d] f32 -> [128, B*nkb, dc, 128] bf16 (SBUF half-chunk layout)."""
    nkb, dc = s // 128, d // 128
    t = x.reshape(B, nkb, 128, dc, 128).transpose(4, 0, 1, 3, 2)
    return np.ascontiguousarray(t.reshape(128, B * nkb, dc, 128)).astype(NPBF16)


def _tile_wq(w, d=D):
    """[d, NH*HD] f32 (already rope-permuted) -> [128, NH, dc, HD] bf16."""
    dc = d // 128
    t = w.reshape(dc, 128, NH, HD).transpose(1, 2, 0, 3)
    return np.ascontiguousarray(t).astype(NPBF16)


def _tile_wkv(w, d=D):
    """[d, HD] f32 -> [128, dc, HD] bf16."""
    dc = d // 128
    return np.ascontiguousarray(w.reshape(dc, 128, HD).transpose(1, 0, 2)).astype(NPBF16)


def _tile_wo(w, d=D):
    """[NH*HD, d] f32 -> [128, d//512, NH, 512] bf16."""
    t = w.reshape(NH, 128, d // 512, 512).transpose(1, 2, 0, 3)
    return np.ascontiguousarray(t).astype(NPBF16)


def _get_program(causal):
    if causal not in _CACHE:
        _CACHE[causal] = build_program(causal)
    return _CACHE[causal]


def _is_causal(mask):
    iu = np.triu_indices(S, 1)
    il = np.tril_indices(S)
    return bool(np.all(mask[il] == 0.0) and np.all(mask[iu] < -1e8))


def make_in_maps(x, cos, sin, mask, wq, wk, wv, wo, causal):
    x = np.asarray(x, dtype=np.float32)
    cos = np.asarray(cos, dtype=np.float32)
    sin = np.asarray(sin, dtype=np.float32)
    mask = np.asarray(mask, dtype=np.float32)
    wq = np.asarray(wq, dtype=np.float32)
    wk = np.asarray(wk, dtype=np.float32)
    wv = np.asarray(wv, dtype=np.float32)
    wo = np.asarray(wo, dtype=np.float32)

    xh = _tile_xh(x)
    c2 = np.concatenate([cos.T, cos.T], 0)
    s2 = np.concatenate([-sin.T, sin.T], 0)
    # fused [cos | sin] bf16 table: [128, 2S], one contiguous 4KB-line DMA
    cs2 = np.ascontiguousarray(np.concatenate([c2, s2], 1)).astype(NPBF16)
    swm = np.eye(128, dtype=np.float32).astype(NPBF16)  # transpose identity
    if causal:
        mt = np.ascontiguousarray(mask[:128, :128].T * math.sqrt(HD)).astype(np.float32)
    else:
        mt = np.ascontiguousarray(mask.T * math.sqrt(HD)).astype(NPBF16)

    in_maps = []
    for c in range(NCORES):
        wq_c = wq[:, c * NH * HD : (c + 1) * NH * HD].reshape(D, NH, HD)[:, :, _PERM]
        wq_c = _tile_wq(wq_c.reshape(D, NH * HD))
        wk_c = _tile_wkv(np.ascontiguousarray(wk[:, c * HD : (c + 1) * HD][:, _PERM]))
        wv_c = _tile_wkv(np.ascontiguousarray(wv[:, c * HD : (c + 1) * HD]))
        wo_c = _tile_wo(np.ascontiguousarray(wo[c * NH * HD : (c + 1) * NH * HD, :]))
        in_maps.append(
            {
                "xh": xh,
                "wq": wq_c,
                "wk": wk_c,
                "wv": wv_c,
                "wo": wo_c,
                "sw": swm,
                "cs": cs2,
                "mt": mt,
            }
        )
    return in_maps


def run(in_maps, causal, **kwargs):
    nc = _get_program(causal)
    return run_bass_kernel_spmd(nc, in_maps, core_ids=list(range(NCORES)), **kwargs)


def kernel(x, start_pos, cos, sin, mask, wq, wk, wv, wo):
    mask = np.asarray(mask, dtype=np.float32)
    causal = _is_causal(mask)
    in_maps = make_in_maps(x, cos, sin, mask, wq, wk, wv, wo, causal)
    res = run(in_maps, causal)
    acc = np.zeros((B * S, D), dtype=np.float32)
    for c in range(NCORES):
        acc += np.asarray(res.results[c]["out"], dtype=np.float32)
    return acc.reshape(B, S, D)



# revision 34
# speedup vs baseline: 1.0438x; 1.0438x over previous
"""GQA attention prefill (B=2, S=1024, D=4096, H=32, KVH=8, HD=128) on 8 TRN2
NeuronCores.

Sharding: tensor-parallel over heads. Core c owns KV head c and query heads
4c..4c+3 (GQA groups align with cores), i.e. column-shards of wq/wk/wv and the
matching row-shard of wo. Each core computes its partial `attn_c @ wo_c`
([B*S, D]); the host sums the 8 partials.

Device inputs are host-pretiled into exact SBUF layouts so every DMA reads
fully contiguous per-partition lines (see _tile_* helpers):
  xh   [128, B*nkb, dc, 128]  bf16  activation half-chunks (contraction on
                                    partitions, 128 tokens per chunk)
  wq   [128, NH, dc, HD]  bf16  rope-permuted (even dims then odd, per head)
  wk   [128, dc, HD]      bf16  rope-permuted
  wv   [128, dc, HD]      bf16
  wo   [128, D/512, NH, 512] bf16
  sw   [128, 128]         bf16  identity (PE-transpose operand for V)
  c2   [128, S]   f32   rope cos table, duplicated across the two 64-row halves
  s2   [128, S]   f32   rope sin table, [-sin; +sin]
  mt   [128,128]  f32   causal diagonal-block additive mask * sqrt(HD)   (causal)
  mt   [S, S]     bf16  full transposed additive mask * sqrt(HD)         (general)

Q/K are produced transposed ([d, tok]) straight out of the projection matmul;
scores are computed transposed ([k, q]) so P^T feeds wide-N PV matmuls
(nested causal ranges) with no transposes anywhere. Softmax denominators:
DVE folds each PAIR of k-blocks of P^T into one tile (halving the columns
the slow [1,N]-shaped ones-matmuls stream through the PE), the per-pair
ones-matmuls accumulate the denominator row in PSUM two chunks late (so they
never head-of-line-block the PE on exp/fold latency), and a separate "den"
unit — placed so zipped filler matmuls run between it and the scores —
evicts the row, broadcasts it across partitions with a rank-1 PE matmul
(gpsimd's partition_broadcast pays a ~3us ucode-reload when alternating with
SWDGE work), and takes the DVE reciprocal. V is projected transposed and
PE-transposed back. Rope's even/odd pairing becomes a contiguous
partition-half swap by permuting the weight columns; the swap is two
SBUF->SBUF DMAs, and the swap/mul/add chain runs at 256-column grain over
per-128 PSUM evictions (interleaving accumulation groups of two column
regions in one PSUM tile diverges on hardware). Softmax skips the
max-subtraction (scores are O(10); exp accumulates in fp32).

Startup: the first matmul needs only half of wk + the first x half-chunk.
x and projection-weight DMAs ride the sync (HWDGE) ring in exact consumption
order (wk.0 | x0 | wk.1, wv | x1 | wq0 | x2 | wq1 | x3 | wq2, wq3 | x4..):
the Tile list scheduler orders ready DMAs by emission priority, so lazy loads
are emitted inside the first unit that needs them, and the big rope tables
are gated behind the first rope eviction (a 1-element WAW write) to keep
them out of the opening window. The first four half-chunks' units are
emitted in a hand-crafted order matched to DMA arrival. wo rides the scalar
ring, gated on the batch's last qT write so the scheduler cannot hoist the
(dependency-free) loads into the startup window. Output stores ride the two
HWDGE rings only.

Scheduling: all work is emitted as unit-closures and "zipped" — batch 1's
projections (at 256-token grain, so the zipped rope/eviction chains stay
coarse) interleave batch 0's attention, batch 0's output projection
interleaves batch 1's attention — so the in-order PE queue always holds
dense matmuls while exp/softmax chains drain on the other engines. PSUM
evictions are emitted one projection late to avoid head-of-line blocking.
"""

import math
from contextlib import ExitStack

import numpy as np
import ml_dtypes

import concourse.bass as bass
import concourse.bass_isa as bass_isa
import concourse.mybir as mybir
import concourse.tile as tile
from concourse import bacc
from concourse.bass_utils import run_bass_kernel_spmd

BF16 = mybir.dt.bfloat16
F32 = mybir.dt.float32
NPBF16 = ml_dtypes.bfloat16

B, S, D, H, KVH, HD = 2, 1024, 4096, 32, 8, 128
NCORES = 8
NH = H // NCORES          # 4 query heads per core
DC = D // 128             # 32 contraction chunks
SQ = 1.0 / math.sqrt(HD)


def _chunks(q0, qend, step=512):
    qs = q0
    while qs < qend:
        nq = min(step, qend - qs)
        yield qs, nq
        qs += nq


def build_program(causal, s=S, d=D):
    """Build the per-core SPMD program. s/d are overridable for sim tests."""
    dc = d // 128
    nkb = s // 128            # number of 128-wide key/query/token blocks per batch
    qcols = NH * HD

    # pT packing offsets: causal keeps only k-block ki's valid q range [128ki, s)
    if causal:
        q0s = [ki * 128 for ki in range(nkb)]
    else:
        q0s = [0] * nkb
    offs, acc = [], 0
    for ki in range(nkb):
        offs.append(acc)
        acc += s - q0s[ki]
    pt_len = acc

    nc = bacc.Bacc(
        "TRN2",
        target_bir_lowering=False,
        debug=False,
        enable_asserts=False,
        num_devices=1,
    )
    # all inputs are host-pretiled into the exact SBUF layouts, so every DMA
    # below reads fully contiguous per-partition lines
    xh = nc.dram_tensor("xh", [128, B * nkb, dc, 128], BF16, kind="ExternalInput").ap()
    wq = nc.dram_tensor("wq", [128, NH, dc, HD], BF16, kind="ExternalInput").ap()
    wk = nc.dram_tensor("wk", [128, dc, HD], BF16, kind="ExternalInput").ap()
    wv = nc.dram_tensor("wv", [128, dc, HD], BF16, kind="ExternalInput").ap()
    wo = nc.dram_tensor("wo", [128, d // 512, NH, 512], BF16, kind="ExternalInput").ap()
    sw = nc.dram_tensor("sw", [128, 128], BF16, kind="ExternalInput").ap()
    cs = nc.dram_tensor("cs", [128, 2 * s], BF16, kind="ExternalInput").ap()
    if causal:
        mt = nc.dram_tensor("mt", [128, 128], F32, kind="ExternalInput").ap()
    else:
        mt = nc.dram_tensor("mt", [s, s], BF16, kind="ExternalInput").ap()
    out = nc.dram_tensor("out", [B * s, d], BF16, kind="ExternalOutput").ap()

    with tile.TileContext(nc) as tc:
        with ExitStack() as ctx:
            const = ctx.enter_context(tc.tile_pool(name="const", bufs=1))
            xpool = ctx.enter_context(tc.tile_pool(name="xpool", bufs=4))
            wopool = ctx.enter_context(tc.tile_pool(name="wopool", bufs=2))
            qkv = ctx.enter_context(tc.tile_pool(name="qkv", bufs=2))
            ptp = ctx.enter_context(tc.tile_pool(name="ptp", bufs=2))
            rp = ctx.enter_context(tc.tile_pool(name="rp", bufs=3))
            small = ctx.enter_context(tc.tile_pool(name="small", bufs=2))
            tre = ctx.enter_context(tc.tile_pool(name="tre", bufs=2)) if causal else None
            oev = ctx.enter_context(tc.tile_pool(name="oev", bufs=2))
            # PSUM: psm (projections/oproj/V-transposes) and psA (attention
            # scores/PV/den-broadcast) are separate pools so a score matmul
            # never waits on a projection eviction queued behind exps on ACT
            # (and vice versa). 3 + 3 banks + 2 for the sums row = 8.
            psm = ctx.enter_context(tc.tile_pool(name="psm", bufs=3, space="PSUM"))
            psA = ctx.enter_context(tc.tile_pool(name="psA", bufs=3, space="PSUM"))
            psd = ctx.enter_context(tc.tile_pool(name="psd", bufs=1, space="PSUM"))

            # Startup streaming is split across BOTH HWDGE rings: the x
            # stream (+wv) rides sync, wk/wq ride scalar (idle at startup).
            # Per-queue throughput is ~550ns per per-partition LINE (16 DMA
            # engines): 2KB lines crawl at ~55GB/s while >=8KB lines stream
            # at full rate — so transfers are never split below 4KB lines,
            # and multi-chunk transfers are fused to widen the lines.
            # wk is issued here as one full-width (8KB-line) transfer.
            wk_sb = const.tile([128, dc, HD], BF16)
            nc.scalar.dma_start(wk_sb[:], wk[:])
            # small tables + the fused bf16 rope table ride the gpsimd queue
            # (separate SWDGE queue at ~75GB/s, out of the HWDGE rings' way)
            id_sb = const.tile([128, 128], BF16)
            nc.gpsimd.dma_start(id_sb[:], sw[:])
            if causal:
                mt_sb = const.tile([128, 128], F32)
                nc.gpsimd.dma_start(mt_sb[:], mt[:])
            else:
                mt_sb = const.tile([128, nkb, s], BF16)
                nc.gpsimd.dma_start(mt_sb[:], mt.rearrange("(kb p) q -> p kb q", p=128))
            # fused [cos | sin] table, bf16: half the bytes of the old f32
            # pair, one 4KB-line DMA, loaded up-front (first rope mul needs
            # it ~16us in; the old lazy f32 load arrived ~9us too late and
            # back-pressured the projection PSUM pool through the rope chain)
            cs_sb = const.tile([128, 2 * s], BF16)
            nc.gpsimd.dma_start(cs_sb[:], cs[:])
            # wv/wq are loaded lazily inside the first units that need them,
            # interleaved into their queues in exact consumption order.
            wv_sb = const.tile([128, dc, HD], BF16)
            wq_sb = const.tile([128, NH, dc, HD], BF16)
            loaded = set()
            ones_sb = const.tile([128, 1], BF16)
            nc.vector.memset(ones_sb[:], 1.0)
            onesr_sb = const.tile([1, 128], BF16)
            nc.vector.memset(onesr_sb[:], 1.0)

            def rope(pieces, tok0, w, out_slice):
                """pieces: per-128 psum tiles with raw projected Q/K columns
                (d-permuted). out = raw*c2 + halfswap(raw)*s2, written as bf16
                to out_slice. Evictions stay at 128-column PSUM grain; the
                swap / mul / add chain runs once per w columns."""
                raw = rp.tile([128, w], BF16, tag="raw", name=f"raw_{tok0}")
                for i, ps in enumerate(pieces):
                    nc.scalar.copy(raw[:, i * 128 : (i + 1) * 128], ps[:])
                swt = rp.tile([128, w], BF16, tag="swt", name=f"swt_{tok0}")
                nc.sync.dma_start(swt[0:64, :], raw[64:128, :])
                nc.sync.dma_start(swt[64:128, :], raw[0:64, :])
                t1 = rp.tile([128, w], F32, tag="t1", name=f"t1_{tok0}")
                nc.vector.tensor_mul(t1[:], swt[:], cs_sb[:, s + tok0 : s + tok0 + w])
                t2 = rp.tile([128, w], F32, tag="t2", name=f"t2_{tok0}")
                nc.vector.tensor_mul(t2[:], raw[:], cs_sb[:, tok0 : tok0 + w])
                nc.gpsimd.tensor_add(out_slice, t2[:], t1[:])

            def phase2(b, grain=1):
                """Stream x half-chunks, project Q/K/V for batch b. Returns
                the per-batch activation tiles. grain = number of 128-token
                halves per unit: batch 0 runs grain=1 for a fine-grained
                startup ramp; batch 1 runs grain=2 so the zipped filler units
                (and their rope/swap chains) stay coarse like the PE work."""
                qT_sb = qkv.tile([128, NH, s], BF16, tag="qT", name=f"qT_{b}")
                kT_sb = qkv.tile([128, s], BF16, tag="kT", name=f"kT_{b}")
                vT_sb = qkv.tile([128, s], BF16, tag="vT", name=f"vT_{b}")
                v_sb = qkv.tile([128, nkb, HD], BF16, tag="v", name=f"v_{b}")
                attnT_sb = qkv.tile([128, NH, s], BF16, tag="attnT", name=f"attnT_{b}")

                # evictions/rope are emitted one projection late, so each
                # engine's FIFO only sees work whose PSUM inputs are (nearly)
                # ready — avoids head-of-line blocking behind matmul chains.
                pending = []

                def flush(keep):
                    while len(pending) > keep:
                        kind, pieces, tok0_, w_ = pending.pop(0)
                        if kind == "k":
                            rope(pieces, tok0_, w_, kT_sb[:, tok0_ : tok0_ + w_])
                        elif kind.startswith("q"):
                            h = int(kind[1:])
                            rope(pieces, tok0_, w_, qT_sb[:, h, tok0_ : tok0_ + w_])
                        else:  # vt
                            for m2 in range(w_ // 128):
                                kb = tok0_ // 128 + m2
                                nc.vector.tensor_copy(
                                    vT_sb[:, kb * 128 : (kb + 1) * 128], pieces[m2][:]
                                )
                                vtp = psm.tile(
                                    [128, HD], BF16, tag="mm", name=f"vtp_{b}_{kb}"
                                )
                                nc.tensor.transpose(
                                    vtp[:], vT_sb[:, kb * 128 : (kb + 1) * 128], id_sb[:]
                                )
                                nc.scalar.copy(v_sb[:, kb, :], vtp[:])

                xcs = {}

                def load_x(js):
                    for j in js:
                        if j not in xcs:
                            xc = xpool.tile(
                                [128, dc, 128], BF16, tag="xc", name=f"xc_{b}_{j}"
                            )
                            nc.sync.dma_start(xc[:], xh[:, b * nkb + j, :, :])
                            xcs[j] = xc

                def proj(w_sb, js, nm):
                    # one PSUM tile + one clean start/stop accumulation chain
                    # per 128-token half (interleaving groups in a shared
                    # tile diverges on hardware)
                    pieces = []
                    for j in js:
                        ps = psm.tile([128, 128], F32, tag="mm", name=f"{nm}_{b}_{j}")
                        for c in range(dc):
                            nc.tensor.matmul(
                                ps[:],
                                w_sb[:, c, :],
                                xcs[j][:, c, :],
                                start=(c == 0),
                                stop=(c == dc - 1),
                            )
                        pieces.append(ps)
                    return pieces

                def mk_k(js):
                    def u():
                        load_x(js)
                        pending.append(("k", proj(wk_sb, js, "kps"), js[0] * 128, 128 * len(js)))
                        flush(1)
                    return u

                def mk_vt(js):
                    def u():
                        # V projection, transposed like K (wide-N matmuls),
                        # then PE-transposed back to natural [tok, d] layout
                        if b == 0 and "wv" not in loaded:
                            loaded.add("wv")
                            nc.sync.dma_start(wv_sb[:], wv[:])
                        pending.append(("vt", proj(wv_sb, js, "vps"), js[0] * 128, 128 * len(js)))
                        flush(1)
                    return u

                def mk_q(js, h):
                    def u():
                        if b == 0 and f"wq{h}" not in loaded:
                            loaded.add(f"wq{h}")
                            nc.scalar.dma_start(wq_sb[:, h, :, :], wq[:, h, :, :])
                        pending.append((f"q{h}", proj(wq_sb[:, h], js, f"qps{h}"), js[0] * 128, 128 * len(js)))
                        flush(1)
                    return u

                mk = {"k": mk_k, "v": mk_vt}

                units = []
                if b == 0 and nkb >= 4:
                    # hand-crafted order for the first four half-chunks,
                    # matched to the dual-ring DMA arrival order:
                    #   sync:   [x0|x1] fused | wv | x2 | x3 ...
                    #   scalar: wk(halves) | wq0 | wq1 | wq2 | wq3 | wo..
                    #   gpsimd: id | mt | cs
                    crafted = [
                        (0, "k"), (1, "k"), (0, "q0"), (1, "q0"), (0, "v"),
                        (1, "v"), (0, "q1"), (1, "q1"), (2, "k"), (2, "v"),
                        (2, "q0"), (3, "k"), (0, "q2"), (1, "q2"), (3, "v"),
                        (2, "q1"), (3, "q0"), (0, "q3"), (1, "q3"), (2, "q2"),
                        (3, "q1"), (2, "q3"), (3, "q2"), (3, "q3"),
                    ]
                    for j, kind in crafted:
                        if kind.startswith("q"):
                            units.append(mk_q([j], int(kind[1:])))
                        else:
                            units.append(mk[kind]([j]))
                    first = 4
                else:
                    first = 0
                for j0 in range(first, nkb, grain):
                    js = list(range(j0, min(j0 + grain, nkb)))
                    units.append(mk_k(js))
                    units.append(mk_vt(js))
                    for h in range(NH):
                        units.append(mk_q(js, h))
                units.append(lambda: flush(0))
                T = dict(qT=qT_sb, kT=kT_sb, vT=vT_sb, v=v_sb, attnT=attnT_sb)
                return T, units

            def attn(b, T):
                """Attention units for batch b, emitted FINE-GRAINED: one
                unit per k-block pair of scores, one per PV q-chunk, one for
                the denominator tail. The zip weaves filler matmul units
                between every one of them, so the exp-gated stretches never
                head-of-line-block the in-order PE queue. PV of head h-1
                interleaves the scores of head h."""
                qT_sb, kT_sb, v_sb, attnT_sb = T["qT"], T["kT"], T["v"], T["attnT"]
                nsub = (nkb + 1) // 2          # scores sub-units per head
                ngroups = nsub if causal else nkb  # sums accumulation groups
                pv_chunks = list(_chunks(0, s))
                state = {}
                stage2 = {}

                def flush_sums(st, keep):
                    sums, psums = st["sums"], st["psums"]
                    while len(psums) > keep:
                        src, p_, a0, a1 = psums.pop(0)
                        nc.tensor.matmul(
                            sums[0:1, a0:a1],
                            ones_sb[:],
                            src(a0, a1),
                            start=(p_ == 0),
                            stop=(p_ == ngroups - 1),
                            skip_group_check=True,
                        )

                def scores_pair(h, p):
                    # scores + exp for k-blocks 2p, 2p+1. DVE folds the pair
                    # of k-blocks of pT into one tile (halving the columns
                    # the slow [1,N]-shaped denominator matmuls stream
                    # through the PE); the sums matmuls run two chunks late
                    # so they never wait on the exps or the fold.
                    if h not in state:
                        state[h] = dict(
                            pT=ptp.tile([128, pt_len], BF16, tag="pt", name=f"pt_{b}_{h}"),
                            sums=psd.tile([1, s], F32, tag="sums", bufs=1, name=f"sums_{b}_{h}"),
                            psums=[],
                        )
                    st = state[h]
                    pT = st["pT"]

                    def pslice(ki, qa, qb):
                        return pT[:, offs[ki] + qa - q0s[ki] : offs[ki] + qb - q0s[ki]]

                    for ki in (2 * p, 2 * p + 1):
                        if ki >= nkb:
                            continue
                        q0 = q0s[ki]
                        for qs_, nq in _chunks(q0, s):
                            sc = psA.tile([128, 512], F32, tag="sc", name=f"sc_{b}_{h}_{ki}_{qs_}")
                            nc.tensor.matmul(
                                sc[:, :nq],
                                kT_sb[:, ki * 128 : (ki + 1) * 128],
                                qT_sb[:, h, qs_ : qs_ + nq],
                                start=True,
                                stop=True,
                            )
                            if causal:
                                if qs_ == q0:  # diagonal block
                                    nc.vector.tensor_add(
                                        sc[:, 0:128], sc[:, 0:128], mt_sb[:]
                                    )
                            else:
                                nc.vector.tensor_add(
                                    sc[:, :nq], sc[:, :nq], mt_sb[:, ki, qs_ : qs_ + nq]
                                )
                            po = offs[ki] + qs_ - q0
                            nc.scalar.activation(
                                pT[:, po : po + nq],
                                sc[:, :nq],
                                mybir.ActivationFunctionType.Exp,
                                scale=SQ,
                            )
                            flush_sums(st, 2)
                        if not causal:
                            # general path: no pair-fold (SBUF for the full
                            # mask instead) — one ones-matmul per k-block
                            src = (lambda k: lambda qa, qb: pslice(k, qa, qb))(ki)
                            a0 = 0
                            while a0 < s:
                                a1 = min((a0 // 512 + 1) * 512, s)
                                st["psums"].append((src, ki, a0, a1))
                                a0 = a1
                            continue
                        if ki % 2 == 1 or ki == nkb - 1:
                            k0 = ki - 1 if ki % 2 == 1 else ki
                            qa0 = q0s[k0]
                            if ki % 2 == 1:
                                qb0 = q0s[ki]
                                s2 = tre.tile(
                                    [128, s - qa0], BF16, tag=f"s2_{p}",
                                    name=f"s2_{b}_{h}_{p}",
                                )
                                if qb0 > qa0:
                                    nc.vector.tensor_copy(
                                        s2[:, 0 : qb0 - qa0], pslice(k0, qa0, qb0)
                                    )
                                nc.vector.tensor_add(
                                    s2[:, qb0 - qa0 :],
                                    pslice(k0, qb0, s),
                                    pslice(ki, qb0, s),
                                )
                                src = (lambda t, o: lambda qa, qb: t[:, qa - o : qb - o])(s2, qa0)
                            else:  # odd tail block folds alone
                                src = (lambda k: lambda qa, qb: pslice(k, qa, qb))(ki)
                            # pieces split at 512-column boundaries so no
                            # matmul output crosses a PSUM bank
                            a0 = qa0
                            while a0 < s:
                                a1 = min((a0 // 512 + 1) * 512, s)
                                st["psums"].append((src, ki // 2, a0, a1))
                                a0 = a1
                    if p == nsub - 1:
                        flush_sums(st, 0)

                def attn_den(h):
                    # denominator tail: PSUM row eviction, rank-1 PE
                    # broadcast (gpsimd's partition_broadcast pays a ~3us
                    # ucode-reload when alternating with SWDGE work), then
                    # DVE reciprocal. A separate unit so zipped fillers sit
                    # between the scores and this chain.
                    st = state.pop(h)
                    pT, sums = st["pT"], st["sums"]
                    nhalf = (s + 511) // 512
                    width = s // nhalf
                    rbrs = []
                    for hs in range(nhalf):
                        rbr = small.tile([128, width], F32, tag="rbr", bufs=4 if causal else 2, name=f"rbr_{b}_{h}_{hs}")
                        ssb = small.tile([1, width], BF16, tag="ssb", bufs=4 if causal else 2, name=f"ssb_{b}_{h}_{hs}")
                        nc.scalar.copy(ssb[0:1, :], sums[0:1, hs * width : (hs + 1) * width])
                        rbp = psA.tile([128, 512], F32, tag="sc", name=f"rbp_{b}_{h}_{hs}")
                        nc.tensor.matmul(
                            rbp[:, :width], onesr_sb[:], ssb[0:1, :],
                            start=True, stop=True,
                        )
                        nc.vector.reciprocal_approx_fast(rbr[:], rbp[:, :width])
                        rbrs.append(rbr)
                    stage2[h] = (pT, rbrs, width)

                def attn_pv(h, ci):
                    # wide-N PV: per q-chunk, each k-block contributes one
                    # matmul over its (nested) valid q range, accumulating in
                    # PSUM — ki==0 always covers the whole chunk, so it opens
                    # the group for every column.
                    pT, rbrs, width = stage2[h]
                    qs0, w = pv_chunks[ci]
                    if ci == len(pv_chunks) - 1:
                        stage2.pop(h)
                    o_ps = psA.tile([128, 512], F32, tag="sc", name=f"ops_{b}_{h}_{ci}")
                    kis = [
                        k for k in range(nkb) if (not causal) or q0s[k] < qs0 + w
                    ]
                    for j, ki in enumerate(kis):
                        qlo = max(q0s[ki], qs0)
                        nc.tensor.matmul(
                            o_ps[:, qlo - qs0 : w],
                            v_sb[:, ki, :],
                            pT[:, offs[ki] + qlo - q0s[ki] : offs[ki] + qs0 + w - q0s[ki]],
                            start=(j == 0),
                            stop=(j == len(kis) - 1),
                            skip_group_check=True,
                        )
                    nc.vector.tensor_mul(
                        attnT_sb[:, h, qs0 : qs0 + w],
                        o_ps[:, :w],
                        rbrs[qs0 // width][:, qs0 % width : qs0 % width + w],
                    )

                units = []
                for h in range(NH):
                    for p in range(nsub):
                        units.append(lambda h=h, p=p: scores_pair(h, p))
                    units.append(lambda h=h: attn_den(h))
                    if h > 0:
                        for ci in range(len(pv_chunks)):
                            units.append(lambda h=h, ci=ci: attn_pv(h - 1, ci))
                for ci in range(len(pv_chunks)):
                    units.append(lambda ci=ci: attn_pv(NH - 1, ci))
                return units

            def oproj(b, T):
                """Output projection units (partial over this core's wo rows).
                wo loads ride the (idle) scalar queue so they never queue
                behind output-store DMAs. A one-element write gated on the
                batch's LAST qT rope pins each load's readiness past the
                startup DMA crunch — the Tile list scheduler would otherwise
                hoist these dependency-free loads to t=0 where they steal
                bandwidth from the opening wk/x transfers."""
                attnT_sb = T["attnT"]
                qT_sb = T["qT"]
                wo_nbs = {}

                def mk(nb, tp):
                    def u():
                        if tp == 0 and nb % 2 == 0:
                            # wo loads fused in PAIRS: 8KB lines stream at
                            # ~210GB/s vs 4KB lines at ~120 — halves the
                            # scalar-queue time and the ACT-engine issue count
                            wo2 = wopool.tile(
                                [128, 2, NH, 512], BF16, tag="wo", name=f"wo_{b}_{nb}"
                            )
                            nc.vector.tensor_copy(
                                wo2[0:1, 0, 0, 0:1], qT_sb[0:1, NH - 1, s - 1 : s]
                            )
                            nc.scalar.dma_start(wo2[:], wo[:, nb : nb + 2, :, :])
                            wo_nbs[nb] = wo2[:, 0]
                            wo_nbs[nb + 1] = wo2[:, 1]
                        ot = oev.tile([128, 2, 512], BF16, tag="ot", bufs=4 if causal else 2, name=f"ot_{b}_{nb}_{tp}")
                        for half in range(2):
                            tbk = tp * 2 + half
                            o2 = psm.tile([128, 512], F32, tag="mm", name=f"o2_{b}_{nb}_{tbk}")
                            for h in range(NH):
                                nc.tensor.matmul(
                                    o2[:],
                                    attnT_sb[:, h, tbk * 128 : (tbk + 1) * 128],
                                    wo_nbs[nb][:, h, :],
                                    start=(h == 0),
                                    stop=(h == NH - 1),
                                )
                            if half == 0:
                                nc.scalar.copy(ot[:, half, :], o2[:])
                            else:
                                nc.vector.tensor_copy(ot[:, half, :], o2[:])
                        # outs rotate over all three rings (gpsimd is idle by
                        # the oproj phases — its rope adds ended with the
                        # projections, and the softmax broadcast rides the PE
                        # — so SWDGE descgen there costs one ucode swap total)
                        ring = [nc.sync, nc.scalar, nc.gpsimd][(nb * 4 + tp) % 3]
                        ring.dma_start(
                            out[
                                b * s + tp * 256 : b * s + (tp + 1) * 256,
                                nb * 512 : (nb + 1) * 512,
                            ].rearrange("(rh p) n -> p rh n", p=128),
                            ot[:],
                        )
                    return u

                return [mk(nb, tp) for nb in range(d // 512) for tp in range(nkb // 2)]

            def zip_emit(primary, filler):
                """Emit primary units with filler units woven between them, so
                the in-order PE queue always has dense matmul work to run
                while the primary's cross-engine chains (exp/softmax) drain.
                Fractional weaving spreads the fillers over ALL primaries."""
                rate = len(filler) / max(1, len(primary))
                credit = 0.0
                fi = 0
                for i, u in enumerate(primary):
                    u()
                    credit += rate
                    while credit >= 1.0 and fi < len(filler):
                        filler[fi]()
                        fi += 1
                        credit -= 1.0
                # remaining fillers drain AFTER the last primary: draining
                # them first serializes their eviction/out-DMA chains in
                # front of the last primary's matmuls (measured ~6us PE gap
                # + a HAM re-throttle)
                while fi < len(filler):
                    filler[fi]()
                    fi += 1

            # batch 0 projections run alone; batch 0 attention is interleaved
            # with batch 1 projections; batch 1 attention with batch 0 output
            # projection; batch 1 output projection drains at the end.
            T0, p20 = phase2(0, grain=2)
            for u in p20:
                u()
            a0 = attn(0, T0)
            if B > 1:
                T1, p21 = phase2(1, grain=2)
                zip_emit(a0, p21)
                a1 = attn(1, T1)
                o0 = oproj(0, T0)
                zip_emit(a1, o0)
                for u in oproj(1, T1):
                    u()
            else:
                for u in a0:
                    u()
                for u in oproj(0, T0):
                    u()
    nc.compile()
    return nc


# ---------------------------------------------------------------------------
# host side
# ---------------------------------------------------------------------------

_PERM = np.concatenate([np.arange(0, HD, 2), np.arange(1, HD, 2)])
_CACHE = {}


def _tile_xh(x, s=S, d=D):
    """[B, s, d] f32 -> [128, B*nkb, dc, 128] bf16 (SBUF half-chunk layout)."""
    nkb, dc = s // 128, d // 128
    t = x.reshape(B, nkb, 128, dc, 128).transpose(4, 0, 1, 3, 2)
    return np.ascontiguousarray(t.reshape(128, B * nkb, dc, 128)).astype(NPBF16)


def _tile_wq(w, d=D):
    """[d, NH*HD] f32 (already rope-permuted) -> [128, NH, dc, HD] bf16."""
    dc = d // 128
    t = w.reshape(dc, 128, NH, HD).transpose(1, 2, 0, 3)
    return np.ascontiguousarray(t).astype(NPBF16)


def _tile_wkv(w, d=D):
    """[d, HD] f32 -> [128, dc, HD] bf16."""
    dc = d // 128
    return np.ascontiguousarray(w.reshape(dc, 128, HD).transpose(1, 0, 2)).astype(NPBF16)


def _tile_wo(w, d=D):
    """[NH*HD, d] f32 -> [128, d//512, NH, 512] bf16."""
    t = w.reshape(NH, 128, d // 512, 512).transpose(1, 2, 0, 3)
    return np.ascontiguousarray(t).astype(NPBF16)


def _get_program(causal):
    if causal not in _CACHE:
        _CACHE[causal] = build_program(causal)
    return _CACHE[causal]


def _is_causal(mask):
    iu = np.triu_indices(S, 1)
    il = np.tril_indices(S)
    return bool(np.all(mask[il] == 0.0) and np.all(mask[iu] < -1e8))


def make_in_maps(x, cos, sin, mask, wq, wk, wv, wo, causal):
    x = np.asarray(x, dtype=np.float32)
    cos = np.asarray(cos, dtype=np.float32)
    sin = np.asarray(sin, dtype=np.float32)
    mask = np.asarray(mask, dtype=np.float32)
    wq = np.asarray(wq, dtype=np.float32)
    wk = np.asarray(wk, dtype=np.float32)
    wv = np.asarray(wv, dtype=np.float32)
    wo = np.asarray(wo, dtype=np.float32)

    xh = _tile_xh(x)
    c2 = np.concatenate([cos.T, cos.T], 0)
    s2 = np.concatenate([-sin.T, sin.T], 0)
    # fused [cos | sin] bf16 table: [128, 2S], one contiguous 4KB-line DMA
    cs2 = np.ascontiguousarray(np.concatenate([c2, s2], 1)).astype(NPBF16)
    swm = np.eye(128, dtype=np.float32).astype(NPBF16)  # transpose identity
    if causal:
        mt = np.ascontiguousarray(mask[:128, :128].T * math.sqrt(HD)).astype(np.float32)
    else:
        mt = np.ascontiguousarray(mask.T * math.sqrt(HD)).astype(NPBF16)

    in_maps = []
    for c in range(NCORES):
        wq_c = wq[:, c * NH * HD : (c + 1) * NH * HD].reshape(D, NH, HD)[:, :, _PERM]
        wq_c = _tile_wq(wq_c.reshape(D, NH * HD))
        wk_c = _tile_wkv(np.ascontiguousarray(wk[:, c * HD : (c + 1) * HD][:, _PERM]))
        wv_c = _tile_wkv(np.ascontiguousarray(wv[:, c * HD : (c + 1) * HD]))
        wo_c = _tile_wo(np.ascontiguousarray(wo[c * NH * HD : (c + 1) * NH * HD, :]))
        in_maps.append(
            {
                "xh": xh,
                "wq": wq_c,
                "wk": wk_c,
                "wv": wv_c,
                "wo": wo_c,
                "sw": swm,
                "cs": cs2,
                "mt": mt,
            }
        )
    return in_maps


def run(in_maps, causal, **kwargs):
    nc = _get_program(causal)
    return run_bass_kernel_spmd(nc, in_maps, core_ids=list(range(NCORES)), **kwargs)


def kernel(x, start_pos, cos, sin, mask, wq, wk, wv, wo):
    mask = np.asarray(mask, dtype=np.float32)
    causal = _is_causal(mask)
    in_maps = make_in_maps(x, cos, sin, mask, wq, wk, wv, wo, causal)
    res = run(in_maps, causal)
    acc = np.zeros((B * S, D), dtype=np.float32)
    for c in range(NCORES):
        acc += np.asarray(res.results[c]["out"], dtype=np.float32)
    return acc.reshape(B, S, D)



# revision 40
# speedup vs baseline: 1.0645x; 1.0198x over previous
"""GQA attention prefill (B=2, S=1024, D=4096, H=32, KVH=8, HD=128) on 8 TRN2
NeuronCores.

Sharding: tensor-parallel over heads. Core c owns KV head c and query heads
4c..4c+3 (GQA groups align with cores), i.e. column-shards of wq/wk/wv and the
matching row-shard of wo. Each core computes its partial `attn_c @ wo_c`
([B*S, D]); the host sums the 8 partials.

Device inputs are host-pretiled into exact SBUF layouts so every DMA reads
fully contiguous per-partition lines (see _tile_* helpers):
  xh   [128, B*nkb, dc, 128]  bf16  activation half-chunks (contraction on
                                    partitions, 128 tokens per chunk)
  wq   [128, NH, dc, HD]  bf16  rope-permuted (even dims then odd, per head)
  wk   [128, dc, HD]      bf16  rope-permuted
  wv   [128, dc, HD]      bf16
  wo   [128, D/512, NH, 512] bf16
  sw   [128, 128]         bf16  identity (PE-transpose operand for V)
  c2   [128, S]   f32   rope cos table, duplicated across the two 64-row halves
  s2   [128, S]   f32   rope sin table, [-sin; +sin]
  mt   [128,128]  f32   causal diagonal-block additive mask * sqrt(HD)   (causal)
  mt   [S, S]     bf16  full transposed additive mask * sqrt(HD)         (general)

Q/K are produced transposed ([d, tok]) straight out of the projection matmul;
scores are computed transposed ([k, q]) so P^T feeds wide-N PV matmuls
(nested causal ranges) with no transposes anywhere. Softmax denominators:
DVE folds each PAIR of k-blocks of P^T into one tile (halving the columns
the slow [1,N]-shaped ones-matmuls stream through the PE), the per-pair
ones-matmuls accumulate the denominator row in PSUM two chunks late (so they
never head-of-line-block the PE on exp/fold latency), and a separate "den"
unit — placed so zipped filler matmuls run between it and the scores —
evicts the row, broadcasts it across partitions with a rank-1 PE matmul
(gpsimd's partition_broadcast pays a ~3us ucode-reload when alternating with
SWDGE work), and takes the DVE reciprocal. V is projected transposed and
PE-transposed back. Rope's even/odd pairing becomes a contiguous
partition-half swap by permuting the weight columns; the swap is two
SBUF->SBUF DMAs, and the swap/mul/add chain runs at 256-column grain over
per-128 PSUM evictions (interleaving accumulation groups of two column
regions in one PSUM tile diverges on hardware). Softmax skips the
max-subtraction (scores are O(10); exp accumulates in fp32).

Startup: the first matmul needs only half of wk + the first x half-chunk.
x and projection-weight DMAs ride the sync (HWDGE) ring in exact consumption
order (wk.0 | x0 | wk.1, wv | x1 | wq0 | x2 | wq1 | x3 | wq2, wq3 | x4..):
the Tile list scheduler orders ready DMAs by emission priority, so lazy loads
are emitted inside the first unit that needs them, and the big rope tables
are gated behind the first rope eviction (a 1-element WAW write) to keep
them out of the opening window. The first four half-chunks' units are
emitted in a hand-crafted order matched to DMA arrival. wo rides the scalar
ring, gated on the batch's last qT write so the scheduler cannot hoist the
(dependency-free) loads into the startup window. Output stores ride the two
HWDGE rings only.

Scheduling: all work is emitted as unit-closures and "zipped" — batch 1's
projections (at 256-token grain, so the zipped rope/eviction chains stay
coarse) interleave batch 0's attention, batch 0's output projection
interleaves batch 1's attention — so the in-order PE queue always holds
dense matmuls while exp/softmax chains drain on the other engines. PSUM
evictions are emitted one projection late to avoid head-of-line blocking.
"""

import math
from contextlib import ExitStack

import numpy as np
import ml_dtypes

import concourse.bass as bass
import concourse.bass_isa as bass_isa
import concourse.mybir as mybir
import concourse.tile as tile
from concourse import bacc
from concourse.bass_utils import run_bass_kernel_spmd

BF16 = mybir.dt.bfloat16
F32 = mybir.dt.float32
NPBF16 = ml_dtypes.bfloat16

B, S, D, H, KVH, HD = 2, 1024, 4096, 32, 8, 128
NCORES = 8
NH = H // NCORES          # 4 query heads per core
DC = D // 128             # 32 contraction chunks
SQ = 1.0 / math.sqrt(HD)


def _chunks(q0, qend, step=512):
    qs = q0
    while qs < qend:
        nq = min(step, qend - qs)
        yield qs, nq
        qs += nq


def build_program(causal, s=S, d=D):
    """Build the per-core SPMD program. s/d are overridable for sim tests."""
    dc = d // 128
    nkb = s // 128            # number of 128-wide key/query/token blocks per batch
    qcols = NH * HD

    # pT packing offsets: causal keeps only k-block ki's valid q range [128ki, s)
    if causal:
        q0s = [ki * 128 for ki in range(nkb)]
    else:
        q0s = [0] * nkb
    offs, acc = [], 0
    for ki in range(nkb):
        offs.append(acc)
        acc += s - q0s[ki]
    pt_len = acc

    nc = bacc.Bacc(
        "TRN2",
        target_bir_lowering=False,
        debug=False,
        enable_asserts=False,
        num_devices=1,
    )
    # all inputs are host-pretiled into the exact SBUF layouts, so every DMA
    # below reads fully contiguous per-partition lines
    xh = nc.dram_tensor("xh", [128, B * nkb, dc, 128], BF16, kind="ExternalInput").ap()
    wq = nc.dram_tensor("wq", [128, NH, dc, HD], BF16, kind="ExternalInput").ap()
    wk = nc.dram_tensor("wk", [128, dc, HD], BF16, kind="ExternalInput").ap()
    wv = nc.dram_tensor("wv", [128, dc, HD], BF16, kind="ExternalInput").ap()
    wo = nc.dram_tensor("wo", [128, d // 512, NH, 512], BF16, kind="ExternalInput").ap()
    sw = nc.dram_tensor("sw", [128, 128], BF16, kind="ExternalInput").ap()
    cs = nc.dram_tensor("cs", [128, 2 * s], BF16, kind="ExternalInput").ap()
    if causal:
        mt = nc.dram_tensor("mt", [128, 128], F32, kind="ExternalInput").ap()
    else:
        mt = nc.dram_tensor("mt", [s, s], BF16, kind="ExternalInput").ap()
    out = nc.dram_tensor("out", [B * s, d], BF16, kind="ExternalOutput").ap()

    with tile.TileContext(nc) as tc:
        with ExitStack() as ctx:
            const = ctx.enter_context(tc.tile_pool(name="const", bufs=1))
            xpool = ctx.enter_context(tc.tile_pool(name="xpool", bufs=4))
            wopool = ctx.enter_context(tc.tile_pool(name="wopool", bufs=2))
            qkv = ctx.enter_context(tc.tile_pool(name="qkv", bufs=2))
            ptp = ctx.enter_context(tc.tile_pool(name="ptp", bufs=2))
            rp = ctx.enter_context(tc.tile_pool(name="rp", bufs=3))
            small = ctx.enter_context(tc.tile_pool(name="small", bufs=2))
            tre = ctx.enter_context(tc.tile_pool(name="tre", bufs=2)) if causal else None
            oev = ctx.enter_context(tc.tile_pool(name="oev", bufs=2))
            # PSUM: psm (projections/oproj/V-transposes) and psA (attention
            # scores/PV/den-broadcast) are separate pools so a score matmul
            # never waits on a projection eviction queued behind exps on ACT
            # (and vice versa). 3 + 3 banks + 2 for the sums row = 8.
            psm = ctx.enter_context(tc.tile_pool(name="psm", bufs=3, space="PSUM"))
            psA = ctx.enter_context(tc.tile_pool(name="psA", bufs=3, space="PSUM"))
            psd = ctx.enter_context(tc.tile_pool(name="psd", bufs=1, space="PSUM"))

            # Startup streaming is split across BOTH HWDGE rings: the x
            # stream (+wv) rides sync, wk/wq ride scalar (idle at startup).
            # Per-queue throughput is ~550ns per per-partition LINE (16 DMA
            # engines): 2KB lines crawl at ~55GB/s while >=8KB lines stream
            # at full rate — so transfers are never split below 4KB lines,
            # and multi-chunk transfers are fused to widen the lines.
            # wk is issued here as one full-width (8KB-line) transfer.
            wk_sb = const.tile([128, dc, HD], BF16)
            nc.scalar.dma_start(wk_sb[:], wk[:])
            # small tables + the fused bf16 rope table ride the gpsimd queue
            # (separate SWDGE queue at ~75GB/s, out of the HWDGE rings' way)
            id_sb = const.tile([128, 128], BF16)
            nc.gpsimd.dma_start(id_sb[:], sw[:])
            if causal:
                mt_sb = const.tile([128, 128], F32)
                nc.gpsimd.dma_start(mt_sb[:], mt[:])
            else:
                mt_sb = const.tile([128, nkb, s], BF16)
                nc.gpsimd.dma_start(mt_sb[:], mt.rearrange("(kb p) q -> p kb q", p=128))
            # fused [cos | sin] table, bf16: half the bytes of the old f32
            # pair, one 4KB-line DMA, loaded up-front (first rope mul needs
            # it ~16us in; the old lazy f32 load arrived ~9us too late and
            # back-pressured the projection PSUM pool through the rope chain)
            cs_sb = const.tile([128, 2 * s], BF16)
            nc.gpsimd.dma_start(cs_sb[:], cs[:])
            # wv/wq are loaded lazily inside the first units that need them,
            # interleaved into their queues in exact consumption order —
            # except wq2, which rides the (otherwise idle) gpsimd queue and
            # is issued HERE so its SWDGE descgen doesn't interleave with
            # the rope tensor_adds (ucode swap)
            wv_sb = const.tile([128, dc, HD], BF16)
            wq_sb = const.tile([128, NH, dc, HD], BF16)
            if NH > 2:
                nc.gpsimd.dma_start(wq_sb[:, 2, :, :], wq[:, 2, :, :])
            loaded = set(["wq2"] if NH > 2 else [])
            ones_sb = const.tile([128, 1], BF16)
            nc.vector.memset(ones_sb[:], 1.0)
            onesr_sb = const.tile([1, 128], BF16)
            nc.vector.memset(onesr_sb[:], 1.0)

            def rope(ps, tok0, w, out_slice):
                """ps: psum tile with raw projected Q/K columns (d-permuted).
                out = raw*c2 + halfswap(raw)*s2, written as bf16 to
                out_slice. One full-width eviction; the swap / mul / add
                chain runs once per w columns."""
                raw = rp.tile([128, w], BF16, tag="raw", name=f"raw_{tok0}")
                nc.scalar.copy(raw[:], ps[:, :w])
                swt = rp.tile([128, w], BF16, tag="swt", name=f"swt_{tok0}")
                nc.sync.dma_start(swt[0:64, :], raw[64:128, :])
                nc.sync.dma_start(swt[64:128, :], raw[0:64, :])
                t1 = rp.tile([128, w], F32, tag="t1", name=f"t1_{tok0}")
                nc.vector.tensor_mul(t1[:], swt[:], cs_sb[:, s + tok0 : s + tok0 + w])
                t2 = rp.tile([128, w], F32, tag="t2", name=f"t2_{tok0}")
                nc.vector.tensor_mul(t2[:], raw[:], cs_sb[:, tok0 : tok0 + w])
                nc.gpsimd.tensor_add(out_slice, t2[:], t1[:])

            def phase2(b, grain=1):
                """Stream x half-chunks, project Q/K/V for batch b. Returns
                the per-batch activation tiles. grain = number of 128-token
                halves per unit: batch 0 runs grain=1 for a fine-grained
                startup ramp; batch 1 runs grain=2 so the zipped filler units
                (and their rope/swap chains) stay coarse like the PE work."""
                qT_sb = qkv.tile([128, NH, s], BF16, tag="qT", name=f"qT_{b}")
                kT_sb = qkv.tile([128, s], BF16, tag="kT", name=f"kT_{b}")
                vT_sb = qkv.tile([128, s], BF16, tag="vT", name=f"vT_{b}")
                v_sb = qkv.tile([128, nkb, HD], BF16, tag="v", name=f"v_{b}")
                attnT_sb = qkv.tile([128, NH, s], BF16, tag="attnT", name=f"attnT_{b}")

                # evictions/rope are emitted one projection late, so each
                # engine's FIFO only sees work whose PSUM inputs are (nearly)
                # ready — avoids head-of-line blocking behind matmul chains.
                pending = []

                def flush(keep):
                    while len(pending) > keep:
                        kind, ps, tok0_, w_ = pending.pop(0)
                        if kind == "k":
                            rope(ps, tok0_, w_, kT_sb[:, tok0_ : tok0_ + w_])
                        elif kind.startswith("q"):
                            h = int(kind[1:])
                            rope(ps, tok0_, w_, qT_sb[:, h, tok0_ : tok0_ + w_])
                        else:  # vt
                            kb0 = tok0_ // 128
                            nc.vector.tensor_copy(
                                vT_sb[:, tok0_ : tok0_ + w_], ps[:, :w_]
                            )
                            vtp = psm.tile(
                                [128, w_], BF16, tag="mm", name=f"vtp_{b}_{kb0}"
                            )
                            for m2 in range(w_ // 128):
                                nc.tensor.transpose(
                                    vtp[:, m2 * 128 : (m2 + 1) * 128],
                                    vT_sb[:, (kb0 + m2) * 128 : (kb0 + m2 + 1) * 128],
                                    id_sb[:],
                                )
                            nc.scalar.copy(
                                v_sb[:, kb0 : kb0 + w_ // 128, :], vtp[:, :w_]
                            )

                xcs = {}

                def load_x(js):
                    for j in js:
                        if j not in xcs:
                            xc = xpool.tile(
                                [128, dc, 128], BF16, tag="xc", name=f"xc_{b}_{j}"
                            )
                            nc.sync.dma_start(xc[:], xh[:, b * nkb + j, :, :])
                            xcs[j] = xc

                def proj(w_sb, js, nm):
                    # SEQUENTIAL accumulation chains into disjoint 128-col
                    # regions of ONE psum tile (a chain's start=True clears
                    # the whole bank's has_written bits but not its data, so
                    # completed regions survive; chains must never
                    # interleave). One tile per unit halves PSUM slot
                    # pressure and enables single wide evictions.
                    ps = psm.tile(
                        [128, 128 * len(js)], F32, tag="mm", name=f"{nm}_{b}_{js[0]}"
                    )
                    for m, j in enumerate(js):
                        for c in range(dc):
                            nc.tensor.matmul(
                                ps[:, m * 128 : (m + 1) * 128],
                                w_sb[:, c, :],
                                xcs[j][:, c, :],
                                start=(c == 0),
                                stop=(c == dc - 1),
                                skip_group_check=True,
                            )
                    return ps

                def mk_k(js):
                    def u():
                        load_x(js)
                        pending.append(("k", proj(wk_sb, js, "kps"), js[0] * 128, 128 * len(js)))
                        flush(1)
                    return u

                def mk_vt(js):
                    def u():
                        # V projection, transposed like K (wide-N matmuls),
                        # then PE-transposed back to natural [tok, d] layout
                        if b == 0 and "wv" not in loaded:
                            loaded.add("wv")
                            nc.sync.dma_start(wv_sb[:], wv[:])
                        pending.append(("vt", proj(wv_sb, js, "vps"), js[0] * 128, 128 * len(js)))
                        flush(1)
                    return u

                def mk_q(js, h):
                    def u():
                        if b == 0 and f"wq{h}" not in loaded:
                            loaded.add(f"wq{h}")
                            # wq0/wq1 ride scalar (behind wk); wq3 rides sync
                            # (behind x3); wq2 was issued up-front on gpsimd —
                            # three queues aggregate ~400GB/s, and no single
                            # queue carries more than ~3MB of the startup load
                            ring = nc.scalar if h < 2 else nc.sync
                            ring.dma_start(wq_sb[:, h, :, :], wq[:, h, :, :])
                        pending.append((f"q{h}", proj(wq_sb[:, h], js, f"qps{h}"), js[0] * 128, 128 * len(js)))
                        flush(1)
                    return u

                mk = {"k": mk_k, "v": mk_vt}

                units = []
                if b == 0 and nkb >= 4:
                    # hand-crafted order for the first four half-chunks,
                    # matched to the dual-ring DMA arrival order:
                    #   sync:   [x0|x1] fused | wv | x2 | x3 ...
                    #   scalar: wk(halves) | wq0 | wq1 | wq2 | wq3 | wo..
                    #   gpsimd: id | mt | cs
                    crafted = [
                        (0, "k"), (1, "k"), (0, "q0"), (1, "q0"), (0, "v"),
                        (1, "v"), (0, "q1"), (1, "q1"), (2, "k"), (2, "v"),
                        (2, "q0"), (3, "k"), (0, "q2"), (1, "q2"), (3, "v"),
                        (2, "q1"), (3, "q0"), (0, "q3"), (1, "q3"), (2, "q2"),
                        (3, "q1"), (2, "q3"), (3, "q2"), (3, "q3"),
                    ]
                    for j, kind in crafted:
                        if kind.startswith("q"):
                            units.append(mk_q([j], int(kind[1:])))
                        else:
                            units.append(mk[kind]([j]))
                    first = 4
                else:
                    first = 0
                for j0 in range(first, nkb, grain):
                    js = list(range(j0, min(j0 + grain, nkb)))
                    units.append(mk_k(js))
                    units.append(mk_vt(js))
                    for h in range(NH):
                        units.append(mk_q(js, h))
                units.append(lambda: flush(0))
                T = dict(qT=qT_sb, kT=kT_sb, vT=vT_sb, v=v_sb, attnT=attnT_sb)
                return T, units

            def attn(b, T):
                """Attention units for batch b, emitted FINE-GRAINED: one
                unit per k-block pair of scores, one per PV q-chunk, one for
                the denominator tail. The zip weaves filler matmul units
                between every one of them, so the exp-gated stretches never
                head-of-line-block the in-order PE queue. PV of head h-1
                interleaves the scores of head h."""
                qT_sb, kT_sb, v_sb, attnT_sb = T["qT"], T["kT"], T["v"], T["attnT"]
                nsub = (nkb + 1) // 2          # scores sub-units per head
                ngroups = nsub if causal else nkb  # sums accumulation groups
                pv_chunks = list(_chunks(0, s))
                state = {}
                stage2 = {}

                def flush_sums(st, keep):
                    sums, psums = st["sums"], st["psums"]
                    while len(psums) > keep:
                        src, p_, a0, a1 = psums.pop(0)
                        nc.tensor.matmul(
                            sums[0:1, a0:a1],
                            ones_sb[:],
                            src(a0, a1),
                            start=(p_ == 0),
                            stop=(p_ == ngroups - 1),
                            skip_group_check=True,
                        )

                def scores_pair(h, p):
                    # scores + exp for k-blocks 2p, 2p+1. DVE folds the pair
                    # of k-blocks of pT into one tile (halving the columns
                    # the slow [1,N]-shaped denominator matmuls stream
                    # through the PE); the sums matmuls run two chunks late
                    # so they never wait on the exps or the fold.
                    if h not in state:
                        state[h] = dict(
                            pT=ptp.tile([128, pt_len], BF16, tag="pt", name=f"pt_{b}_{h}"),
                            sums=psd.tile([1, s], F32, tag="sums", bufs=1, name=f"sums_{b}_{h}"),
                            psums=[],
                        )
                    st = state[h]
                    pT = st["pT"]

                    def pslice(ki, qa, qb):
                        return pT[:, offs[ki] + qa - q0s[ki] : offs[ki] + qb - q0s[ki]]

                    for ki in (2 * p, 2 * p + 1):
                        if ki >= nkb:
                            continue
                        q0 = q0s[ki]
                        for qs_, nq in _chunks(q0, s):
                            sc = psA.tile([128, 512], F32, tag="sc", name=f"sc_{b}_{h}_{ki}_{qs_}")
                            nc.tensor.matmul(
                                sc[:, :nq],
                                kT_sb[:, ki * 128 : (ki + 1) * 128],
                                qT_sb[:, h, qs_ : qs_ + nq],
                                start=True,
                                stop=True,
                            )
                            if causal:
                                if qs_ == q0:  # diagonal block
                                    nc.vector.tensor_add(
                                        sc[:, 0:128], sc[:, 0:128], mt_sb[:]
                                    )
                            else:
                                nc.vector.tensor_add(
                                    sc[:, :nq], sc[:, :nq], mt_sb[:, ki, qs_ : qs_ + nq]
                                )
                            po = offs[ki] + qs_ - q0
                            nc.scalar.activation(
                                pT[:, po : po + nq],
                                sc[:, :nq],
                                mybir.ActivationFunctionType.Exp,
                                scale=SQ,
                            )
                            flush_sums(st, 2)
                        if not causal:
                            # general path: no pair-fold (SBUF for the full
                            # mask instead) — one ones-matmul per k-block
                            src = (lambda k: lambda qa, qb: pslice(k, qa, qb))(ki)
                            a0 = 0
                            while a0 < s:
                                a1 = min((a0 // 512 + 1) * 512, s)
                                st["psums"].append((src, ki, a0, a1))
                                a0 = a1
                            continue
                        if ki % 2 == 1 or ki == nkb - 1:
                            k0 = ki - 1 if ki % 2 == 1 else ki
                            qa0 = q0s[k0]
                            if ki % 2 == 1:
                                qb0 = q0s[ki]
                                s2 = tre.tile(
                                    [128, s - qa0], BF16, tag=f"s2_{p}",
                                    name=f"s2_{b}_{h}_{p}",
                                )
                                if qb0 > qa0:
                                    nc.vector.tensor_copy(
                                        s2[:, 0 : qb0 - qa0], pslice(k0, qa0, qb0)
                                    )
                                nc.vector.tensor_add(
                                    s2[:, qb0 - qa0 :],
                                    pslice(k0, qb0, s),
                                    pslice(ki, qb0, s),
                                )
                                src = (lambda t, o: lambda qa, qb: t[:, qa - o : qb - o])(s2, qa0)
                            else:  # odd tail block folds alone
                                src = (lambda k: lambda qa, qb: pslice(k, qa, qb))(ki)
                            # pieces split at 512-column boundaries so no
                            # matmul output crosses a PSUM bank
                            a0 = qa0
                            while a0 < s:
                                a1 = min((a0 // 512 + 1) * 512, s)
                                st["psums"].append((src, ki // 2, a0, a1))
                                a0 = a1
                    if p == nsub - 1:
                        flush_sums(st, 0)

                def attn_den(h):
                    # denominator tail: PSUM row eviction, rank-1 PE
                    # broadcast (gpsimd's partition_broadcast pays a ~3us
                    # ucode-reload when alternating with SWDGE work), then
                    # DVE reciprocal. A separate unit so zipped fillers sit
                    # between the scores and this chain.
                    st = state.pop(h)
                    pT, sums = st["pT"], st["sums"]
                    nhalf = (s + 511) // 512
                    width = s // nhalf
                    rbrs = []
                    for hs in range(nhalf):
                        rbr = small.tile([128, width], F32, tag="rbr", bufs=4 if causal else 2, name=f"rbr_{b}_{h}_{hs}")
                        ssb = small.tile([1, width], BF16, tag="ssb", bufs=4 if causal else 2, name=f"ssb_{b}_{h}_{hs}")
                        nc.scalar.copy(ssb[0:1, :], sums[0:1, hs * width : (hs + 1) * width])
                        rbp = psA.tile([128, 512], F32, tag="sc", name=f"rbp_{b}_{h}_{hs}")
                        nc.tensor.matmul(
                            rbp[:, :width], onesr_sb[:], ssb[0:1, :],
                            start=True, stop=True,
                        )
                        nc.vector.reciprocal_approx_fast(rbr[:], rbp[:, :width])
                        rbrs.append(rbr)
                    stage2[h] = (pT, rbrs, width)

                def attn_pv(h, ci):
                    # wide-N PV: per q-chunk, each k-block contributes one
                    # matmul over its (nested) valid q range, accumulating in
                    # PSUM — ki==0 always covers the whole chunk, so it opens
                    # the group for every column.
                    pT, rbrs, width = stage2[h]
                    qs0, w = pv_chunks[ci]
                    if ci == len(pv_chunks) - 1:
                        stage2.pop(h)
                    o_ps = psA.tile([128, 512], F32, tag="sc", name=f"ops_{b}_{h}_{ci}")
                    kis = [
                        k for k in range(nkb) if (not causal) or q0s[k] < qs0 + w
                    ]
                    for j, ki in enumerate(kis):
                        qlo = max(q0s[ki], qs0)
                        nc.tensor.matmul(
                            o_ps[:, qlo - qs0 : w],
                            v_sb[:, ki, :],
                            pT[:, offs[ki] + qlo - q0s[ki] : offs[ki] + qs0 + w - q0s[ki]],
                            start=(j == 0),
                            stop=(j == len(kis) - 1),
                            skip_group_check=True,
                        )
                    nc.vector.tensor_mul(
                        attnT_sb[:, h, qs0 : qs0 + w],
                        o_ps[:, :w],
                        rbrs[qs0 // width][:, qs0 % width : qs0 % width + w],
                    )

                units = []
                for h in range(NH):
                    for p in range(nsub):
                        units.append(lambda h=h, p=p: scores_pair(h, p))
                    units.append(lambda h=h: attn_den(h))
                    if h > 0:
                        for ci in range(len(pv_chunks)):
                            units.append(lambda h=h, ci=ci: attn_pv(h - 1, ci))
                for ci in range(len(pv_chunks)):
                    units.append(lambda ci=ci: attn_pv(NH - 1, ci))
                return units

            def oproj(b, T):
                """Output projection units (partial over this core's wo rows).
                wo loads ride the (idle) scalar queue so they never queue
                behind output-store DMAs. A one-element write gated on the
                batch's LAST qT rope pins each load's readiness past the
                startup DMA crunch — the Tile list scheduler would otherwise
                hoist these dependency-free loads to t=0 where they steal
                bandwidth from the opening wk/x transfers."""
                attnT_sb = T["attnT"]
                qT_sb = T["qT"]
                wo_nbs = {}

                def mk(nb, tp):
                    def u():
                        if tp == 0 and nb % 2 == 0:
                            # wo loads fused in PAIRS: 8KB lines stream at
                            # ~210GB/s vs 4KB lines at ~120 — halves the
                            # scalar-queue time and the ACT-engine issue count
                            wo2 = wopool.tile(
                                [128, 2, NH, 512], BF16, tag="wo", name=f"wo_{b}_{nb}"
                            )
                            nc.vector.tensor_copy(
                                wo2[0:1, 0, 0, 0:1], qT_sb[0:1, NH - 1, s - 1 : s]
                            )
                            nc.scalar.dma_start(wo2[:], wo[:, nb : nb + 2, :, :])
                            wo_nbs[nb] = wo2[:, 0]
                            wo_nbs[nb + 1] = wo2[:, 1]
                        ot = oev.tile([128, 2, 512], BF16, tag="ot", bufs=4 if causal else 2, name=f"ot_{b}_{nb}_{tp}")
                        for half in range(2):
                            tbk = tp * 2 + half
                            o2 = psm.tile([128, 512], F32, tag="mm", name=f"o2_{b}_{nb}_{tbk}")
                            for h in range(NH):
                                nc.tensor.matmul(
                                    o2[:],
                                    attnT_sb[:, h, tbk * 128 : (tbk + 1) * 128],
                                    wo_nbs[nb][:, h, :],
                                    start=(h == 0),
                                    stop=(h == NH - 1),
                                )
                            if half == 0:
                                nc.scalar.copy(ot[:, half, :], o2[:])
                            else:
                                nc.vector.tensor_copy(ot[:, half, :], o2[:])
                        # outs rotate over all three rings (gpsimd is idle by
                        # the oproj phases — its rope adds ended with the
                        # projections, and the softmax broadcast rides the PE
                        # — so SWDGE descgen there costs one ucode swap total)
                        ring = [nc.sync, nc.scalar, nc.gpsimd][(nb * 4 + tp) % 3]
                        ring.dma_start(
                            out[
                                b * s + tp * 256 : b * s + (tp + 1) * 256,
                                nb * 512 : (nb + 1) * 512,
                            ].rearrange("(rh p) n -> p rh n", p=128),
                            ot[:],
                        )
                    return u

                return [mk(nb, tp) for nb in range(d // 512) for tp in range(nkb // 2)]

            def zip_emit(primary, filler):
                """Emit primary units with filler units woven between them, so
                the in-order PE queue always has dense matmul work to run
                while the primary's cross-engine chains (exp/softmax) drain.
                Fractional weaving spreads the fillers over ALL primaries."""
                rate = len(filler) / max(1, len(primary))
                credit = 0.0
                fi = 0
                for i, u in enumerate(primary):
                    u()
                    credit += rate
                    while credit >= 1.0 and fi < len(filler):
                        filler[fi]()
                        fi += 1
                        credit -= 1.0
                # remaining fillers drain AFTER the last primary: draining
                # them first serializes their eviction/out-DMA chains in
                # front of the last primary's matmuls (measured ~6us PE gap
                # + a HAM re-throttle)
                while fi < len(filler):
                    filler[fi]()
                    fi += 1

            # batch 0 projections run alone; batch 0 attention is interleaved
            # with batch 1 projections; batch 1 attention with batch 0 output
            # projection; batch 1 output projection drains at the end.
            T0, p20 = phase2(0, grain=2)
            for u in p20:
                u()
            a0 = attn(0, T0)
            if B > 1:
                T1, p21 = phase2(1, grain=2)
                zip_emit(a0, p21)
                a1 = attn(1, T1)
                o0 = oproj(0, T0)
                zip_emit(a1, o0)
                for u in oproj(1, T1):
                    u()
            else:
                for u in a0:
                    u()
                for u in oproj(0, T0):
                    u()
    nc.compile()
    return nc


# ---------------------------------------------------------------------------
# host side
# ---------------------------------------------------------------------------

_PERM = np.concatenate([np.arange(0, HD, 2), np.arange(1, HD, 2)])
_CACHE = {}


def _tile_xh(x, s=S, d=D):
    """[B, s, d] f32 -> [128, B*nkb, dc, 128] bf16 (SBUF half-chunk layout)."""
    nkb, dc = s // 128, d // 128
    t = x.reshape(B, nkb, 128, dc, 128).transpose(4, 0, 1, 3, 2)
    return np.ascontiguousarray(t.reshape(128, B * nkb, dc, 128)).astype(NPBF16)


def _tile_wq(w, d=D):
    """[d, NH*HD] f32 (already rope-permuted) -> [128, NH, dc, HD] bf16."""
    dc = d // 128
    t = w.reshape(dc, 128, NH, HD).transpose(1, 2, 0, 3)
    return np.ascontiguousarray(t).astype(NPBF16)


def _tile_wkv(w, d=D):
    """[d, HD] f32 -> [128, dc, HD] bf16."""
    dc = d // 128
    return np.ascontiguousarray(w.reshape(dc, 128, HD).transpose(1, 0, 2)).astype(NPBF16)


def _tile_wo(w, d=D):
    """[NH*HD, d] f32 -> [128, d//512, NH, 512] bf16."""
    t = w.reshape(NH, 128, d // 512, 512).transpose(1, 2, 0, 3)
    return np.ascontiguousarray(t).astype(NPBF16)


def _get_program(causal):
    if causal not in _CACHE:
        _CACHE[causal] = build_program(causal)
    return _CACHE[causal]


def _is_causal(mask):
    iu = np.triu_indices(S, 1)
    il = np.tril_indices(S)
    return bool(np.all(mask[il] == 0.0) and np.all(mask[iu] < -1e8))


def make_in_maps(x, cos, sin, mask, wq, wk, wv, wo, causal):
    x = np.asarray(x, dtype=np.float32)
    cos = np.asarray(cos, dtype=np.float32)
    sin = np.asarray(sin, dtype=np.float32)
    mask = np.asarray(mask, dtype=np.float32)
    wq = np.asarray(wq, dtype=np.float32)
    wk = np.asarray(wk, dtype=np.float32)
    wv = np.asarray(wv, dtype=np.float32)
    wo = np.asarray(wo, dtype=np.float32)

    xh = _tile_xh(x)
    c2 = np.concatenate([cos.T, cos.T], 0)
    s2 = np.concatenate([-sin.T, sin.T], 0)
    # fused [cos | sin] bf16 table: [128, 2S], one contiguous 4KB-line DMA
    cs2 = np.ascontiguousarray(np.concatenate([c2, s2], 1)).astype(NPBF16)
    swm = np.eye(128, dtype=np.float32).astype(NPBF16)  # transpose identity
    if causal:
        mt = np.ascontiguousarray(mask[:128, :128].T * math.sqrt(HD)).astype(np.float32)
    else:
        mt = np.ascontiguousarray(mask.T * math.sqrt(HD)).astype(NPBF16)

    in_maps = []
    for c in range(NCORES):
        wq_c = wq[:, c * NH * HD : (c + 1) * NH * HD].reshape(D, NH, HD)[:, :, _PERM]
        wq_c = _tile_wq(wq_c.reshape(D, NH * HD))
        wk_c = _tile_wkv(np.ascontiguousarray(wk[:, c * HD : (c + 1) * HD][:, _PERM]))
        wv_c = _tile_wkv(np.ascontiguousarray(wv[:, c * HD : (c + 1) * HD]))
        wo_c = _tile_wo(np.ascontiguousarray(wo[c * NH * HD : (c + 1) * NH * HD, :]))
        in_maps.append(
            {
                "xh": xh,
                "wq": wq_c,
                "wk": wk_c,
                "wv": wv_c,
                "wo": wo_c,
                "sw": swm,
                "cs": cs2,
                "mt": mt,
            }
        )
    return in_maps


def run(in_maps, causal, **kwargs):
    nc = _get_program(causal)
    return run_bass_kernel_spmd(nc, in_maps, core_ids=list(range(NCORES)), **kwargs)


def kernel(x, start_pos, cos, sin, mask, wq, wk, wv, wo):
    mask = np.asarray(mask, dtype=np.float32)
    causal = _is_causal(mask)
    in_maps = make_in_maps(x, cos, sin, mask, wq, wk, wv, wo, causal)
    res = run(in_maps, causal)
    acc = np.zeros((B * S, D), dtype=np.float32)
    for c in range(NCORES):
        acc += np.asarray(res.results[c]["out"], dtype=np.float32)
    return acc.reshape(B, S, D)

